# revision 1
# baseline (speedup 1.0000x reference)
"""Multi-head attention on 8 Trainium2 NeuronCores — fp16 pipeline.

Sharding: tensor-parallel over heads (2 heads/core), full batch on every
core; host sums the 8 partial outputs and adds b_o + b_v @ w_o.

vs the fp32r version: all matmul operands are fp16 (1 cycle/row, FWL
fast weight loads, 1024-wide moving operand, 2-byte DMA-xbar transpose
for x^T). fp32r matmuls are self-loading (one serialized ~208 ns
LDWEIGHTS per matmul) which caps them at ~2x slower in practice.

Per core, per batch bi:
  A: xT[kt] [128, S] fp16 <- DMA-xbar-transpose of x16[bi, :, kt*128:...]
  B: Q^T/K^T = w^T xT + b (PSUM f32, bias fused in DVE copy -> fp16)
     V natural [tok, 128] via lhsT = xT tile, rhs = w_v tile; DVE
     strided-copy into vt chunks [V_A | 1 | V_B | 1] per 130 cols.
  C: per head, per 1024-wide q-chunk: 17-step software pipeline
     kt:   S^T MM [128k, 1024q] -> PSUM   (lhsT = K^T tile [64, 128])
           stage-D fill for batch bi-1 (2 MMs + 2 DVE + DMA per unit)
     kt-1: ACT exp(PSUM * 0.125) -> pexp fp16
           AV MM [65, 1024] accumulate (lhsT = vt chunk [128, 65])
     row 64 of AV = softmax denominators r; 1/r via PE transpose +
     DVE reciprocal -> per-partition scalars [128, n_tt].
  D (interleaved into C of batch bi+1): per (tt, half):
     po_h [128, 512] = attnT_h^T @ w_o_h; DVE: t1 = po_0 * rrec0;
     osb = (po_1 * rrec1) + t1 -> fp16 -> DMA out.
"""

import numpy as np

import concourse.bacc as bacc
import concourse.mybir as mybir
from concourse.tile import TileContext
from concourse.masks import make_identity
from concourse import bass_utils

dt = mybir.dt
F32 = dt.float32
F16 = dt.float16
AF = mybir.ActivationFunctionType
ALU = mybir.AluOpType

B, S, D = 4, 2048, 1024
H, DH = 16, 64
NCORES = 8
HPC = H // NCORES          # heads per core = 2
DHC = HPC * DH             # 128 projection cols per core

_CACHE = {}


def build_nc(b=B, s=S):
    d = D
    n_tt = s // 128            # token tiles per batch
    n_kt = d // 128            # contraction tiles for projections
    qw = 1024 if s >= 1024 else s
    n_jc = s // qw
    assert s % 512 == 0 and d == 1024

    nc = bacc.Bacc("TRN2", target_bir_lowering=False, debug=False)

    x_d = nc.dram_tensor("x", [b, d, s], F16, kind="ExternalInput")
    wq_d = nc.dram_tensor("wq", [d, DHC], F16, kind="ExternalInput")
    wk_d = nc.dram_tensor("wk", [d, DHC], F16, kind="ExternalInput")
    wv_d = nc.dram_tensor("wv", [d, DHC], F16, kind="ExternalInput")
    bq_d = nc.dram_tensor("bq", [DHC, 1], F32, kind="ExternalInput")
    bk_d = nc.dram_tensor("bk", [DHC, 1], F32, kind="ExternalInput")
    wo_d = nc.dram_tensor("wo", [DHC, d], F16, kind="ExternalInput")
    out_d = nc.dram_tensor("out", [b, s, d], F16, kind="ExternalOutput")
    rs_d = nc.dram_tensor("rscratch", [b, HPC, s], F32)

    with TileContext(nc) as tc:
        with (
            tc.tile_pool(name="const", bufs=1) as cpool,
            tc.tile_pool(name="wpool", bufs=3 * n_kt) as wpool,
            tc.tile_pool(name="xt", bufs=2 * n_kt) as xt_pool,
            tc.tile_pool(name="qk", bufs=2) as qk_pool,
            tc.tile_pool(name="vt", bufs=3) as vt_pool,
            tc.tile_pool(name="at", bufs=2) as at_pool,
            tc.tile_pool(name="pexp", bufs=3) as pexp_pool,
            tc.tile_pool(name="rline", bufs=2) as rline_pool,
            tc.tile_pool(name="small", bufs=6) as small,
            tc.tile_pool(name="osb", bufs=4) as osb_pool,
            tc.tile_pool(name="ps", bufs=1, space="PSUM") as pp,
        ):
            # ---- constants & weights ----
            ident = cpool.tile([128, 128], F32, tag="ident")
            make_identity(nc, ident[:, :])
            ones_col = cpool.tile([128, 32], F16, tag="ones_col")
            nc.vector.memset(ones_col[:, :], 1.0)

            bq = cpool.tile([DHC, 1], F32, tag="bq")
            bk = cpool.tile([DHC, 1], F32, tag="bk")
            nc.sync.dma_start(out=bq[:, :], in_=bq_d[:, :])
            nc.sync.dma_start(out=bk[:, :], in_=bk_d[:, :])

            w16 = {}
            for name, dram in (("q", wq_d), ("k", wk_d), ("v", wv_d)):
                for kt in range(n_kt):
                    wt = wpool.tile([128, DHC], F16, tag="w",
                                    name=f"w_{name}{kt}")
                    nc.sync.dma_start(
                        out=wt[:, :], in_=dram[kt * 128:(kt + 1) * 128, :]
                    )
                    w16[(name, kt)] = wt
            wo = cpool.tile([DHC, d], F16, tag="wo")
            nc.sync.dma_start(out=wo[:, :], in_=wo_d[:, :])

            # stage-D state carried across the batch loop
            prev_d = None   # (attnT, rrec) of previous batch
            d_queue = []    # pending stage-D units for prev batch

            def emit_d_unit(bi_out):
                """Emit one (tt, half) output unit of the previous batch."""
                if not d_queue:
                    return
                attnT_p, rrec_p, tt, half = d_queue.pop(0)
                cs = slice(half * 512, (half + 1) * 512)
                poA = pp.tile([128, 512], F32, tag="poA", name="poA")
                poB = pp.tile([128, 512], F32, tag="poB", name="poB")
                nc.tensor.matmul(
                    poA[:, :], attnT_p[0:64, tt * 128:(tt + 1) * 128],
                    wo[0:64, cs], start=True, stop=True,
                    tile_position=(0, 0),
                )
                nc.tensor.matmul(
                    poB[:, :], attnT_p[64:128, tt * 128:(tt + 1) * 128],
                    wo[64:128, cs], start=True, stop=True,
                    tile_position=(64, 0),
                )
                t1 = osb_pool.tile([128, 512], F32, tag="t1", name="t1")
                nc.vector.tensor_scalar_mul(
                    t1[:, :], poA[:, :], rrec_p[0][:, tt:tt + 1]
                )
                osb = osb_pool.tile([128, 512], F16, tag="osb", name="osb")
                nc.vector.scalar_tensor_tensor(
                    osb[:, :], poB[:, :], rrec_p[1][:, tt:tt + 1], t1[:, :],
                    ALU.mult, ALU.add,
                )
                nc.sync.dma_start(
                    out=out_d[bi_out, tt * 128:(tt + 1) * 128, cs],
                    in_=osb[:, :],
                )

            for bi in range(b):
                # ---- stage A: x^T via DMA xbar transpose ----
                xT = [xt_pool.tile([128, s], F16, tag="xt", name=f"xT{kt}")
                      for kt in range(n_kt)]
                for kt in range(n_kt):
                    nc.sync.dma_start(
                        out=xT[kt][:, :],
                        in_=x_d[bi, kt * 128:(kt + 1) * 128, :],
                    )

                # ---- stage B: Q^T, K^T projections ----
                qT = qk_pool.tile([DHC, s], F16, tag="qT")
                kT = qk_pool.tile([DHC, s], F16, tag="kT")
                for name, dst, bias in (("q", qT, bq), ("k", kT, bk)):
                    for c in range(s // qw):
                        ppr = pp.tile([128, qw], F32, tag="st", bufs=2,
                                      name="ppr")
                        for kt in range(n_kt):
                            for j in range(qw // 512):
                                nc.tensor.matmul(
                                    ppr[:, j * 512:(j + 1) * 512],
                                    w16[(name, kt)][:, :],
                                    xT[kt][:, c * qw + j * 512:
                                            c * qw + (j + 1) * 512],
                                    start=(kt == 0),
                                    stop=(kt == n_kt - 1),
                                )
                        nc.vector.tensor_scalar_add(
                            dst[:, c * qw:(c + 1) * qw], ppr[:, :], bias[:, 0:1]
                        )
                # V natural, interleaved-head layout [V_A |1| V_B |1] per 130
                vt = vt_pool.tile([128, n_tt * 130], F16, tag="vt")
                ones_dst = vt.rearrange("p (t two sv) -> p t two sv",
                                        two=2, sv=65)[:, :, :, 64]
                nc.vector.tensor_copy(ones_dst, ones_col[:, 0:2 * n_tt]
                                      .rearrange("p (t two) -> p t two", two=2))
                for tt in range(n_tt):
                    pv = pp.tile([128, 128], F32, tag="poA", name="pv")
                    for kt in range(n_kt):
                        nc.tensor.matmul(
                            pv[:, :],
                            xT[kt][:, tt * 128:(tt + 1) * 128],
                            w16[("v", kt)][:, :],
                            start=(kt == 0),
                            stop=(kt == n_kt - 1),
                        )
                    vdst = vt.rearrange("p (t two sv) -> p t two sv",
                                        two=2, sv=65)[:, tt, :, 0:64]
                    nc.vector.tensor_copy(
                        vdst, pv.rearrange("p (two sv) -> p two sv", two=2)
                    )

                # ---- stage C: attention (+ interleaved stage D of bi-1) ----
                attnT = at_pool.tile([DHC, s], F16, tag="attnT")
                rrec = [small.tile([128, n_tt], F32, tag="rrec",
                                   name=f"rrec{h}") for h in range(HPC)]
                vtv = vt.rearrange("p (t two sv) -> p t two sv", two=2, sv=65)
                for h in range(HPC):
                    hs = slice(h * 64, (h + 1) * 64)
                    rline = rline_pool.tile([1, s], F32, tag="rline")
                    for jc in range(n_jc):
                        qs = slice(jc * qw, (jc + 1) * qw)
                        av = pp.tile([65, qw], F32, tag="av", name="av")
                        pexps = {}
                        for kt in range(n_tt + 1):
                            if kt < n_tt:
                                st = pp.tile([128, qw], F32, tag="st",
                                             bufs=2, name="st")
                                for j in range(qw // 512):
                                    nc.tensor.matmul(
                                        st[:, j * 512:(j + 1) * 512],
                                        kT[hs, kt * 128:(kt + 1) * 128],
                                        qT[hs, jc * qw + j * 512:
                                           jc * qw + (j + 1) * 512],
                                        start=True, stop=True,
                                    )
                                emit_d_unit(bi - 1)
                                pexp = pexp_pool.tile([128, qw], F16,
                                                      tag="pexp", name="pexp")
                                nc.scalar.activation(
                                    pexp[:, :], st[:, :], AF.Exp, scale=0.125
                                )
                                pexps[kt] = pexp
                            if kt > 0:
                                px = pexps.pop(kt - 1)
                                for j in range(qw // 512):
                                    nc.tensor.matmul(
                                        av[:, j * 512:(j + 1) * 512],
                                        vtv[:, kt - 1, h, :],
                                        px[:, j * 512:(j + 1) * 512],
                                        start=(kt == 1),
                                        stop=(kt == n_tt),
                                    )
                        nc.vector.tensor_copy(attnT[hs, qs], av[0:64, :])
                        nc.vector.tensor_copy(
                            rline[0:1, qs], av[64:65, :]
                        )
                    # 1/r -> per-partition scalars via DRAM roundtrip
                    nc.sync.dma_start(
                        out=rs_d[bi, h, :].rearrange("(a c) -> a c", a=1),
                        in_=rline[0:1, :],
                    )
                    r16 = small.tile([n_tt, 128], F32, tag="r16")
                    nc.sync.dma_start(
                        out=r16[:, :],
                        in_=rs_d[bi, h, :].rearrange("(a c) -> a c", a=n_tt),
                    )
                    prt = pp.tile([128, n_tt], F32, tag="av", name="prt")
                    nc.tensor.transpose(
                        prt[:, :], r16[:, :], ident[0:n_tt, 0:n_tt]
                    )
                    nc.vector.reciprocal(rrec[h][:, :], prt[:, :])

                # queue stage D for this batch
                prev_d = (attnT, rrec)
                for tt in range(n_tt):
                    for half in range(2):
                        d_queue.append((attnT, rrec, tt, half))

            # flush the last batch's stage D
            while d_queue:
                emit_d_unit(b - 1)

    nc.compile()
    return nc


def _get_nc(b, s):
    key = (b, s)
    if key not in _CACHE:
        _CACHE[key] = build_nc(b, s)
    return _CACHE[key]


def make_in_maps(x, w_q, b_q, w_k, b_k, w_v, w_o):
    x16 = np.ascontiguousarray(
        np.asarray(x, dtype=np.float16).transpose(0, 2, 1))
    wq16 = np.asarray(w_q, dtype=np.float16)
    wk16 = np.asarray(w_k, dtype=np.float16)
    wv16 = np.asarray(w_v, dtype=np.float16)
    wo16 = np.asarray(w_o, dtype=np.float16)
    in_maps = []
    for i in range(NCORES):
        cs = slice(i * DHC, (i + 1) * DHC)
        in_maps.append({
            "x": x16,
            "wq": np.ascontiguousarray(wq16[:, cs]),
            "wk": np.ascontiguousarray(wk16[:, cs]),
            "wv": np.ascontiguousarray(wv16[:, cs]),
            "bq": np.ascontiguousarray(b_q[cs, None], dtype=np.float32),
            "bk": np.ascontiguousarray(b_k[cs, None], dtype=np.float32),
            "wo": np.ascontiguousarray(wo16[cs, :]),
        })
    return in_maps


def kernel(x, w_q, b_q, w_k, b_k, w_v, b_v, w_o, b_o, _trace=False):
    x = np.asarray(x, dtype=np.float32)
    nc = _get_nc(x.shape[0], x.shape[1])
    in_maps = make_in_maps(x, w_q, b_q, w_k, b_k, w_v, w_o)
    kw = {}
    if _trace:
        import tempfile
        kw = dict(trace=True, trace_cores=list(range(NCORES)),
                  tmpdir=tempfile.mkdtemp(prefix="mha_trace_"))
    res = bass_utils.run_bass_kernel_spmd(
        nc, in_maps, core_ids=list(range(NCORES)), **kw
    )
    out = np.zeros(x.shape, dtype=np.float32)
    for i in range(NCORES):
        out += np.asarray(res.results[i]["out"], dtype=np.float32)
    out += np.asarray(b_o, dtype=np.float32)[None, None, :]
    out += (np.asarray(b_v, dtype=np.float32)
            @ np.asarray(w_o, dtype=np.float32))[None, None, :]
    if _trace:
        return out, res
    return out



# revision 3
# speedup vs baseline: 1.0362x; 1.0362x over previous
"""Multi-head attention on 8 Trainium2 NeuronCores — ACT-paced fp16 pipeline.

Sharding: tensor-parallel over heads (2 heads/core), full batch on every
core; host sums the 8 partial outputs and adds b_o + b_v @ w_o.

Key structure (vs the previous version): the scalar engine's exp is the
irreducible floor (~73 us/batch at N=1024 per ACTIVATE), so the whole
kernel is paced by it and every other engine hides underneath:

  - Scores: 2-head row-tiled matmul pairs (tile_position (0,0)/(64,0))
    into one [128, 1024] f32 PSUM pair-tile (h0 cols 0:512 -> bank A,
    h1 cols 512:1024 -> bank B), double-buffered. One exp per kt step
    covers both heads.
  - b_k is dropped: softmax is invariant to per-q constants, and the
    only score term that varies over k is (q + b_q)@k. b_q is added to
    qT at evacuation (exact same math as the reference).
  - Softmax denominators: av row 64 (ones-row trick) -> DVE reciprocal
    [1, 512] -> gpsimd partition_broadcast -> attnT is normalized once
    at evacuation. Stage D needs no per-unit scaling.
  - Stage D per (tt, half): two concurrent row-tiled MMs -> one DVE
    tensor_add (cast fp16) -> DMA out.
  - Stage B of batch bi+1 (projections) is interleaved into stage C of
    bi via a work queue, so ACT never idles at batch boundaries.

PSUM: stp 2x2 banks + av 2x1 + scratch 2x1 (po pairs / ppr / pv) = 8.
"""

import numpy as np

import concourse.bacc as bacc
import concourse.mybir as mybir
from concourse.tile import TileContext
from concourse import bass_utils

dt = mybir.dt
F32 = dt.float32
F16 = dt.float16
AF = mybir.ActivationFunctionType
ALU = mybir.AluOpType

B, S, D = 4, 2048, 1024
H, DH = 16, 64
NCORES = 8
HPC = H // NCORES          # heads per core = 2
DHC = HPC * DH             # 128 projection cols per core

_CACHE = {}


def build_nc(b=B, s=S):
    d = D
    n_tt = s // 128            # 128-token tiles (k tiles, and q out-tiles)
    n_kt = d // 128            # contraction tiles for projections
    qw = 512                   # q-chunk width in stage C
    n_jc = s // qw
    assert s % 512 == 0 and d == 1024

    nc = bacc.Bacc("TRN2", target_bir_lowering=False, debug=False)

    x_d = nc.dram_tensor("x", [b, d, s], F16, kind="ExternalInput")
    wq_d = nc.dram_tensor("wq", [d, DHC], F16, kind="ExternalInput")
    wk_d = nc.dram_tensor("wk", [d, DHC], F16, kind="ExternalInput")
    wv_d = nc.dram_tensor("wv", [d, DHC], F16, kind="ExternalInput")
    bq_d = nc.dram_tensor("bq", [DHC, 1], F32, kind="ExternalInput")
    wo_d = nc.dram_tensor("wo", [DHC, d], F16, kind="ExternalInput")
    out_d = nc.dram_tensor("out", [b, s, d], F16, kind="ExternalOutput")

    with TileContext(nc) as tc:
        with (
            tc.tile_pool(name="const", bufs=1) as cpool,
            tc.tile_pool(name="wpool", bufs=3 * n_kt) as wpool,
            tc.tile_pool(name="xt", bufs=2 * n_kt) as xt_pool,
            tc.tile_pool(name="qk", bufs=4) as qk_pool,
            tc.tile_pool(name="vt", bufs=2) as vt_pool,
            tc.tile_pool(name="at", bufs=2) as at_pool,
            tc.tile_pool(name="pexp", bufs=3) as pexp_pool,
            tc.tile_pool(name="small", bufs=8) as small,
            tc.tile_pool(name="osb", bufs=3) as osb_pool,
            tc.tile_pool(name="ps", bufs=1, space="PSUM") as pp,
        ):
            # ---- constants & weights ----
            ones_col = cpool.tile([128, 32], F16, tag="ones_col")
            nc.vector.memset(ones_col[:, :], 1.0)
            bq = cpool.tile([DHC, 1], F32, tag="bq")
            nc.sync.dma_start(out=bq[:, :], in_=bq_d[:, :])

            w16 = {}
            for name, dram in (("q", wq_d), ("k", wk_d), ("v", wv_d)):
                for kt in range(n_kt):
                    wt = wpool.tile([128, DHC], F16, tag="w",
                                    name=f"w_{name}{kt}")
                    nc.sync.dma_start(
                        out=wt[:, :], in_=dram[kt * 128:(kt + 1) * 128, :]
                    )
                    w16[(name, kt)] = wt
            wo = cpool.tile([DHC, d], F16, tag="wo")
            nc.sync.dma_start(out=wo[:, :], in_=wo_d[:, :])

            # ---------- stage B builders (projections for one batch) ----
            def xT_dma(bi, xT):
                for kt in range(n_kt):
                    nc.sync.dma_start(
                        out=xT[kt][:, :],
                        in_=x_d[bi, kt * 128:(kt + 1) * 128, :],
                    )

            def qk_burst(name, c, xT, dst):
                """One 512-col chunk of the Q^T or K^T projection."""
                cs = slice(c * 512, (c + 1) * 512)
                ppr = pp.tile([128, 512], F32, tag="scr", bufs=2, name="ppr")
                for kt in range(n_kt):
                    nc.tensor.matmul(
                        ppr[:, :], w16[(name, kt)][:, :], xT[kt][:, cs],
                        start=(kt == 0), stop=(kt == n_kt - 1),
                    )
                if name == "q":
                    nc.vector.tensor_scalar_add(dst[:, cs], ppr[:, :],
                                                bq[:, 0:1])
                else:
                    nc.vector.tensor_copy(dst[:, cs], ppr[:, :])

            def v_burst(tt, xT, vt):
                """One 128-token tile of V in natural [tok, dh] layout."""
                pv = pp.tile([128, 128], F32, tag="scr", bufs=2, name="pv")
                for kt in range(n_kt):
                    nc.tensor.matmul(
                        pv[:, :], xT[kt][:, tt * 128:(tt + 1) * 128],
                        w16[("v", kt)][:, :],
                        start=(kt == 0), stop=(kt == n_kt - 1),
                    )
                vdst = vt.rearrange("p (t two sv) -> p t two sv",
                                    two=2, sv=65)[:, tt, :, 0:64]
                nc.vector.tensor_copy(
                    vdst, pv.rearrange("p (two sv) -> p two sv", two=2)
                )

            def ones_fill(vt):
                ones_dst = vt.rearrange("p (t two sv) -> p t two sv",
                                        two=2, sv=65)[:, :, :, 64]
                nc.vector.tensor_copy(
                    ones_dst,
                    ones_col[:, 0:2 * n_tt]
                    .rearrange("p (t two) -> p t two", two=2))

            def make_b_work(xT, qT, kT, vt):
                work = []
                for name, dst in (("q", qT), ("k", kT)):
                    for c in range(s // 512):
                        work.append(lambda n=name, c=c, xT=xT, dst=dst:
                                    qk_burst(n, c, xT, dst))
                for tt in range(n_tt):
                    work.append(lambda tt=tt, xT=xT, vt=vt:
                                v_burst(tt, xT, vt))
                return work

            # ---------- stage D (output projection units) ----------
            d_queue = []   # (attnT, bi, tt, half)

            def emit_d_unit():
                if not d_queue:
                    return
                attnT_p, bi_out, tt, half = d_queue.pop(0)
                cs = slice(half * 512, (half + 1) * 512)
                po = pp.tile([128, 512], F32, tag="scr", bufs=2, name="po")
                nc.tensor.matmul(
                    po[:, :], attnT_p[:, tt * 128:(tt + 1) * 128],
                    wo[:, cs], start=True, stop=True,
                )
                osb = osb_pool.tile([128, 512], F16, tag="osb", name="osb")
                nc.vector.tensor_copy(osb[:, :], po[:, :])
                nc.sync.dma_start(
                    out=out_d[bi_out, tt * 128:(tt + 1) * 128, cs],
                    in_=osb[:, :],
                )

            # ---------- prologue: batch 0 stage A+B ----------
            def new_bufs():
                xT = [xt_pool.tile([128, s], F16, tag="xt", name=f"xT{kt}")
                      for kt in range(n_kt)]
                qT = qk_pool.tile([DHC, s], F16, tag="qT")
                kT = qk_pool.tile([DHC, s], F16, tag="kT")
                vt = vt_pool.tile([128, n_tt * 130], F16, tag="vt")
                return xT, qT, kT, vt

            cur = new_bufs()
            xT_dma(0, cur[0])
            ones_fill(cur[3])
            for it in make_b_work(*cur):
                it()

            # ---------- main loop ----------
            for bi in range(b):
                xT, qT, kT, vt = cur
                if bi + 1 < b:
                    nxt = new_bufs()
                    b_work = make_b_work(*nxt)
                else:
                    nxt = None
                    b_work = []

                attnT = at_pool.tile([DHC, s], F16, tag="attnT")
                vtv = vt.rearrange("p (t two sv) -> p t two sv",
                                   two=2, sv=65)

                for jc in range(n_jc):
                    qs = slice(jc * qw, (jc + 1) * qw)
                    av = [pp.tile([65, qw], F32, tag="av", bufs=2,
                                  name=f"av{h}") for h in range(HPC)]
                    stps = {}
                    pexps = {}
                    for kt in range(n_tt + 1):
                        step = jc * (n_tt + 1) + kt
                        if kt < n_tt:
                            # score pair: h0 -> cols 0:512 (bank A),
                            # h1 -> cols 512:1024 (bank B)
                            stp = pp.tile([128, 2 * qw], F32, tag="stp",
                                          bufs=2, name="stp")
                            ks = slice(kt * 128, (kt + 1) * 128)
                            nc.tensor.matmul(
                                stp[:, 0:qw], kT[0:64, ks], qT[0:64, qs],
                                start=True, stop=True, tile_position=(0, 0),
                            )
                            nc.tensor.matmul(
                                stp[:, qw:2 * qw], kT[64:128, ks],
                                qT[64:128, qs],
                                start=True, stop=True, tile_position=(64, 0),
                            )
                            stps[kt] = stp
                        # interleave: stage D / stage B of next batch
                        if step % 2 == 0:
                            emit_d_unit()
                        elif step == 1:
                            if nxt is not None:
                                xT_dma(bi + 1, nxt[0])
                                ones_fill(nxt[3])
                        elif step >= 13 and b_work:
                            b_work.pop(0)()
                        if kt < n_tt:
                            # exp for both heads in one ACTIVATE
                            pexp = pexp_pool.tile([128, 2 * qw], F16,
                                                  tag="pexp", name="pexp")
                            nc.scalar.activation(
                                pexp[:, :], stps[kt][:, :], AF.Exp,
                                scale=0.125,
                            )
                            pexps[kt] = pexp
                        if kt > 0:
                            px = pexps.pop(kt - 1)
                            stps.pop(kt - 1, None)
                            for h in range(HPC):
                                nc.tensor.matmul(
                                    av[h][:, :], vtv[:, kt - 1, h, :],
                                    px[:, h * qw:(h + 1) * qw],
                                    start=(kt == 1), stop=(kt == n_tt),
                                )
                    # evacuate: normalize attnT by softmax denominators
                    for h in range(HPC):
                        hs = slice(h * 64, (h + 1) * 64)
                        recip = small.tile([1, qw], F32, tag="recip",
                                           name="recip")
                        nc.vector.reciprocal(recip[:, :], av[h][64:65, :])
                        rb = small.tile([64, qw], F32, tag="rb", name="rb")
                        nc.gpsimd.partition_broadcast(rb[:, :], recip[0:1, :])
                        nc.vector.tensor_mul(attnT[hs, qs], av[h][0:64, :],
                                             rb[:, :])
                    # queue stage D for this q-chunk (4 tt x 2 halves)
                    for tt in range(jc * (qw // 128), (jc + 1) * (qw // 128)):
                        for half in range(2):
                            d_queue.append((attnT, bi, tt, half))

                # any stage-B stragglers for the next batch
                while b_work:
                    b_work.pop(0)()
                cur = nxt

            # flush remaining stage D
            while d_queue:
                emit_d_unit()

    nc.compile()
    return nc


def _get_nc(b, s):
    key = (b, s)
    if key not in _CACHE:
        _CACHE[key] = build_nc(b, s)
    return _CACHE[key]


def make_in_maps(x, w_q, b_q, w_k, w_v, w_o):
    x16 = np.ascontiguousarray(
        np.asarray(x, dtype=np.float16).transpose(0, 2, 1))
    wq16 = np.asarray(w_q, dtype=np.float16)
    wk16 = np.asarray(w_k, dtype=np.float16)
    wv16 = np.asarray(w_v, dtype=np.float16)
    wo16 = np.asarray(w_o, dtype=np.float16)
    in_maps = []
    for i in range(NCORES):
        cs = slice(i * DHC, (i + 1) * DHC)
        in_maps.append({
            "x": x16,
            "wq": np.ascontiguousarray(wq16[:, cs]),
            "wk": np.ascontiguousarray(wk16[:, cs]),
            "wv": np.ascontiguousarray(wv16[:, cs]),
            "bq": np.ascontiguousarray(b_q[cs, None], dtype=np.float32),
            "wo": np.ascontiguousarray(wo16[cs, :]),
        })
    return in_maps


def kernel(x, w_q, b_q, w_k, b_k, w_v, b_v, w_o, b_o, _trace=False):
    x = np.asarray(x, dtype=np.float32)
    nc = _get_nc(x.shape[0], x.shape[1])
    in_maps = make_in_maps(x, w_q, b_q, w_k, w_v, w_o)
    kw = {}
    if _trace:
        import tempfile
        kw = dict(trace=True, trace_cores=list(range(NCORES)),
                  tmpdir=tempfile.mkdtemp(prefix="mha_trace_"))
    res = bass_utils.run_bass_kernel_spmd(
        nc, in_maps, core_ids=list(range(NCORES)), **kw
    )
    out = np.zeros(x.shape, dtype=np.float32)
    for i in range(NCORES):
        out += np.asarray(res.results[i]["out"], dtype=np.float32)
    out += np.asarray(b_o, dtype=np.float32)[None, None, :]
    # b_k cancels in softmax (constant per q); b_v @ w_o added here
    out += (np.asarray(b_v, dtype=np.float32)
            @ np.asarray(w_o, dtype=np.float32))[None, None, :]
    if _trace:
        return out, res
    return out


# revision 8
# speedup vs baseline: 1.2660x; 1.2219x over previous
"""Multi-head attention on 8 Trainium2 NeuronCores — ACT-paced fp16 pipeline.

Sharding: tensor-parallel over heads (2 heads/core), full batch on every
core; host sums the 8 partial outputs and adds b_o + b_v @ w_o.

Key structure (vs the previous version): the scalar engine's exp is the
irreducible floor (~73 us/batch at N=1024 per ACTIVATE), so the whole
kernel is paced by it and every other engine hides underneath:

  - Scores: 2-head row-tiled matmul pairs (tile_position (0,0)/(64,0))
    into one [128, 1024] f32 PSUM pair-tile (h0 cols 0:512 -> bank A,
    h1 cols 512:1024 -> bank B), double-buffered. One exp per kt step
    covers both heads.
  - b_k is dropped: softmax is invariant to per-q constants, and the
    only score term that varies over k is (q + b_q)@k. b_q is added to
    qT at evacuation (exact same math as the reference).
  - Softmax denominators: av row 64 (ones-row trick) -> DVE reciprocal
    [1, 512] -> gpsimd partition_broadcast -> attnT is normalized once
    at evacuation. Stage D needs no per-unit scaling.
  - Stage D per (tt, half): two concurrent row-tiled MMs -> one DVE
    tensor_add (cast fp16) -> DMA out.
  - Stage B of batch bi+1 (projections) is interleaved into stage C of
    bi via a work queue, so ACT never idles at batch boundaries.

PSUM: stp 2x2 banks + av 2x1 + scratch 2x1 (po pairs / ppr / pv) = 8.
"""

import numpy as np

import concourse.bacc as bacc
import concourse.mybir as mybir
from concourse.tile import TileContext
from concourse import bass_utils

dt = mybir.dt
F32 = dt.float32
F16 = dt.float16
AF = mybir.ActivationFunctionType
ALU = mybir.AluOpType

B, S, D = 4, 2048, 1024
H, DH = 16, 64
NCORES = 8
HPC = H // NCORES          # heads per core = 2
DHC = HPC * DH             # 128 projection cols per core

_CACHE = {}


def build_nc(b=B, s=S):
    d = D
    n_tt = s // 128            # 128-token tiles (k tiles, and q out-tiles)
    n_kt = d // 128            # contraction tiles for projections
    qw = 512                   # q-chunk width in stage C
    n_jc = s // qw
    assert s % 512 == 0 and d == 1024

    nc = bacc.Bacc("TRN2", target_bir_lowering=False, debug=False)

    x_d = nc.dram_tensor("x", [b, d, s], F16, kind="ExternalInput")
    wq_d = nc.dram_tensor("wq", [d, DHC], F16, kind="ExternalInput")
    wk_d = nc.dram_tensor("wk", [d, DHC], F16, kind="ExternalInput")
    wv_d = nc.dram_tensor("wv", [d, DHC], F16, kind="ExternalInput")
    bq_d = nc.dram_tensor("bq", [DHC, 1], F32, kind="ExternalInput")
    wo_d = nc.dram_tensor("wo", [DHC, d], F16, kind="ExternalInput")
    out_d = nc.dram_tensor("out", [b, s, d], F16, kind="ExternalOutput")

    with TileContext(nc) as tc:
        with (
            tc.tile_pool(name="const", bufs=1) as cpool,
            tc.tile_pool(name="wpool", bufs=3 * n_kt) as wpool,
            tc.tile_pool(name="xt", bufs=2 * n_kt) as xt_pool,
            tc.tile_pool(name="qk", bufs=4) as qk_pool,
            tc.tile_pool(name="vt", bufs=2) as vt_pool,
            tc.tile_pool(name="at", bufs=2) as at_pool,
            tc.tile_pool(name="pexp", bufs=3) as pexp_pool,
            tc.tile_pool(name="small", bufs=8) as small,
            tc.tile_pool(name="osb", bufs=3) as osb_pool,
            tc.tile_pool(name="ps", bufs=1, space="PSUM") as pp,
        ):
            # ---- constants & weights ----
            ones_col = cpool.tile([128, 32], F16, tag="ones_col")
            nc.vector.memset(ones_col[:, :], 1.0)
            bq = cpool.tile([DHC, 1], F32, tag="bq")
            nc.sync.dma_start(out=bq[:, :], in_=bq_d[:, :])

            w16 = {}
            for name, dram in (("q", wq_d), ("k", wk_d), ("v", wv_d)):
                for kt in range(n_kt):
                    wt = wpool.tile([128, DHC], F16, tag="w",
                                    name=f"w_{name}{kt}")
                    nc.sync.dma_start(
                        out=wt[:, :], in_=dram[kt * 128:(kt + 1) * 128, :]
                    )
                    w16[(name, kt)] = wt
            wo = cpool.tile([DHC, d], F16, tag="wo")
            nc.sync.dma_start(out=wo[:, :], in_=wo_d[:, :])

            # ---------- stage B builders (projections for one batch) ----
            def xT_dma(bi, xT):
                for kt in range(n_kt):
                    nc.sync.dma_start(
                        out=xT[kt][:, :],
                        in_=x_d[bi, kt * 128:(kt + 1) * 128, :],
                    )

            def qk_burst(name, c, xT, dst):
                """One 512-col chunk of the Q^T or K^T projection."""
                cs = slice(c * 512, (c + 1) * 512)
                ppr = pp.tile([128, 512], F32, tag="scr", bufs=2, name="ppr")
                for kt in range(n_kt):
                    nc.tensor.matmul(
                        ppr[:, :], w16[(name, kt)][:, :], xT[kt][:, cs],
                        start=(kt == 0), stop=(kt == n_kt - 1),
                    )
                if name == "q":
                    nc.vector.tensor_scalar_add(dst[:, cs], ppr[:, :],
                                                bq[:, 0:1])
                else:
                    nc.vector.tensor_copy(dst[:, cs], ppr[:, :])

            def v_burst(tt, xT, vt):
                """One 128-token tile of V in natural [tok, dh] layout."""
                pv = pp.tile([128, 128], F32, tag="scr", bufs=2, name="pv")
                for kt in range(n_kt):
                    nc.tensor.matmul(
                        pv[:, :], xT[kt][:, tt * 128:(tt + 1) * 128],
                        w16[("v", kt)][:, :],
                        start=(kt == 0), stop=(kt == n_kt - 1),
                    )
                vdst = vt.rearrange("p (t two sv) -> p t two sv",
                                    two=2, sv=65)[:, tt, :, 0:64]
                nc.vector.tensor_copy(
                    vdst, pv.rearrange("p (two sv) -> p two sv", two=2)
                )

            def ones_fill(vt):
                ones_dst = vt.rearrange("p (t two sv) -> p t two sv",
                                        two=2, sv=65)[:, :, :, 64]
                nc.vector.tensor_copy(
                    ones_dst,
                    ones_col[:, 0:2 * n_tt]
                    .rearrange("p (t two) -> p t two", two=2))

            def make_b_work(xT, qT, kT, vt):
                work = []
                for name, dst in (("q", qT), ("k", kT)):
                    for c in range(s // 512):
                        work.append(lambda n=name, c=c, xT=xT, dst=dst:
                                    qk_burst(n, c, xT, dst))
                for tt in range(n_tt):
                    work.append(lambda tt=tt, xT=xT, vt=vt:
                                v_burst(tt, xT, vt))
                return work

            # ---------- stage D (output projection units) ----------
            # d_stage holds the most recent q-chunk's units; they move to
            # d_queue one jc later so the PE never waits on the (slow-ish)
            # reciprocal -> broadcast -> normalize evacuation chain.
            d_queue = []   # (attnT, bi, tt, half)
            d_stage = []

            def emit_d_unit():
                if not d_queue:
                    return
                attnT_p, bi_out, tt, half = d_queue.pop(0)
                cs = slice(half * 512, (half + 1) * 512)
                po = pp.tile([128, 512], F32, tag="scr", bufs=2, name="po")
                nc.tensor.matmul(
                    po[:, :], attnT_p[:, tt * 128:(tt + 1) * 128],
                    wo[:, cs], start=True, stop=True,
                )
                osb = osb_pool.tile([128, 512], F16, tag="osb", name="osb")
                nc.vector.tensor_copy(osb[:, :], po[:, :])
                nc.sync.dma_start(
                    out=out_d[bi_out, tt * 128:(tt + 1) * 128, cs],
                    in_=osb[:, :],
                )

            # ---------- prologue: batch 0 stage A+B ----------
            def new_bufs():
                xT = [xt_pool.tile([128, s], F16, tag="xt", name=f"xT{kt}")
                      for kt in range(n_kt)]
                qT = qk_pool.tile([DHC, s], F16, tag="qT")
                kT = qk_pool.tile([DHC, s], F16, tag="kT")
                vt = vt_pool.tile([128, n_tt * 130], F16, tag="vt")
                return xT, qT, kT, vt

            cur = new_bufs()
            xT_dma(0, cur[0])
            ones_fill(cur[3])
            for it in make_b_work(*cur):
                it()

            # ---------- main loop ----------
            for bi in range(b):
                xT, qT, kT, vt = cur
                if bi + 1 < b:
                    nxt = new_bufs()
                    b_work = make_b_work(*nxt)
                else:
                    nxt = None
                    b_work = []

                attnT = at_pool.tile([DHC, s], F16, tag="attnT")
                vtv = vt.rearrange("p (t two sv) -> p t two sv",
                                   two=2, sv=65)

                for jc in range(n_jc):
                    qs = slice(jc * qw, (jc + 1) * qw)
                    av = [pp.tile([65, qw], F32, tag="av", bufs=2,
                                  name=f"av{h}") for h in range(HPC)]
                    stps = {}
                    pexps = {}
                    for kt in range(n_tt + 1):
                        step = jc * (n_tt + 1) + kt
                        if kt < n_tt:
                            # score pair: h0 -> cols 0:512 (bank A),
                            # h1 -> cols 512:1024 (bank B)
                            stp = pp.tile([128, 2 * qw], F32, tag="stp",
                                          bufs=2, name="stp")
                            ks = slice(kt * 128, (kt + 1) * 128)
                            nc.tensor.matmul(
                                stp[:, 0:qw], kT[0:64, ks], qT[0:64, qs],
                                start=True, stop=True, tile_position=(0, 0),
                            )
                            nc.tensor.matmul(
                                stp[:, qw:2 * qw], kT[64:128, ks],
                                qT[64:128, qs],
                                start=True, stop=True, tile_position=(64, 0),
                            )
                            stps[kt] = stp
                        # interleave: stage D / stage B of next batch
                        if step % 2 == 0:
                            emit_d_unit()
                        elif step == 1:
                            if nxt is not None:
                                xT_dma(bi + 1, nxt[0])
                                ones_fill(nxt[3])
                        elif step >= 13 and b_work:
                            b_work.pop(0)()
                        if kt < n_tt:
                            # exp for both heads in one ACTIVATE
                            pexp = pexp_pool.tile([128, 2 * qw], F16,
                                                  tag="pexp", name="pexp")
                            nc.scalar.activation(
                                pexp[:, :], stps[kt][:, :], AF.Exp,
                                scale=0.125,
                            )
                            pexps[kt] = pexp
                        if kt > 0:
                            px = pexps.pop(kt - 1)
                            stps.pop(kt - 1, None)
                            for h in range(HPC):
                                nc.tensor.matmul(
                                    av[h][:, :], vtv[:, kt - 1, h, :],
                                    px[:, h * qw:(h + 1) * qw],
                                    start=(kt == 1), stop=(kt == n_tt),
                                )
                    # evacuate: normalize attnT by softmax denominators
                    for h in range(HPC):
                        hs = slice(h * 64, (h + 1) * 64)
                        # reciprocal_approx_fast mishandles base_partition>0
                        # inputs; stage row 64 to partition 0 first
                        rrow = small.tile([1, qw], F32, tag="rrow",
                                          name="rrow")
                        nc.vector.tensor_copy(rrow[:, :], av[h][64:65, :])
                        recip = small.tile([1, qw], F32, tag="recip",
                                           name="recip")
                        nc.vector.reciprocal_approx_fast(recip[:, :],
                                                         rrow[:, :])
                        rb = small.tile([64, qw], F32, tag="rb", name="rb")
                        nc.gpsimd.partition_broadcast(rb[:, :], recip[0:1, :])
                        nc.vector.tensor_mul(attnT[hs, qs], av[h][0:64, :],
                                             rb[:, :])
                    # stage stage-D units for this q-chunk (4 tt x 2 halves);
                    # they become eligible one jc later
                    d_queue.extend(d_stage)
                    d_stage = []
                    for tt in range(jc * (qw // 128), (jc + 1) * (qw // 128)):
                        for half in range(2):
                            d_stage.append((attnT, bi, tt, half))

                # any stage-B stragglers for the next batch
                while b_work:
                    b_work.pop(0)()
                cur = nxt

            # flush remaining stage D
            d_queue.extend(d_stage)
            d_stage = []
            while d_queue:
                emit_d_unit()

    nc.compile()
    return nc


def _get_nc(b, s):
    key = (b, s)
    if key not in _CACHE:
        _CACHE[key] = build_nc(b, s)
    return _CACHE[key]


def make_in_maps(x, w_q, b_q, w_k, w_v, w_o):
    x16 = np.ascontiguousarray(
        np.asarray(x, dtype=np.float16).transpose(0, 2, 1))
    wq16 = np.asarray(w_q, dtype=np.float16)
    wk16 = np.asarray(w_k, dtype=np.float16)
    wv16 = np.asarray(w_v, dtype=np.float16)
    wo16 = np.asarray(w_o, dtype=np.float16)
    in_maps = []
    for i in range(NCORES):
        cs = slice(i * DHC, (i + 1) * DHC)
        in_maps.append({
            "x": x16,
            "wq": np.ascontiguousarray(wq16[:, cs]),
            "wk": np.ascontiguousarray(wk16[:, cs]),
            "wv": np.ascontiguousarray(wv16[:, cs]),
            "bq": np.ascontiguousarray(b_q[cs, None], dtype=np.float32),
            "wo": np.ascontiguousarray(wo16[cs, :]),
        })
    return in_maps


def kernel(x, w_q, b_q, w_k, b_k, w_v, b_v, w_o, b_o, _trace=False):
    x = np.asarray(x, dtype=np.float32)
    nc = _get_nc(x.shape[0], x.shape[1])
    in_maps = make_in_maps(x, w_q, b_q, w_k, w_v, w_o)
    kw = {}
    if _trace:
        import tempfile
        kw = dict(trace=True, trace_cores=list(range(NCORES)),
                  tmpdir=tempfile.mkdtemp(prefix="mha_trace_"))
    res = bass_utils.run_bass_kernel_spmd(
        nc, in_maps, core_ids=list(range(NCORES)), **kw
    )
    out = np.zeros(x.shape, dtype=np.float32)
    for i in range(NCORES):
        out += np.asarray(res.results[i]["out"], dtype=np.float32)
    out += np.asarray(b_o, dtype=np.float32)[None, None, :]
    # b_k cancels in softmax (constant per q); b_v @ w_o added here
    out += (np.asarray(b_v, dtype=np.float32)
            @ np.asarray(w_o, dtype=np.float32))[None, None, :]
    if _trace:
        return out, res
    return out


# revision 16
# speedup vs baseline: 1.2951x; 1.0230x over previous
"""Multi-head attention on 8 Trainium2 NeuronCores — ACT-paced fp16 pipeline.

Sharding: tensor-parallel over heads (2 heads/core), full batch on every
core; host sums the 8 partial outputs and adds b_o + b_v @ w_o.

Key structure (vs the previous version): the scalar engine's exp is the
irreducible floor (~73 us/batch at N=1024 per ACTIVATE), so the whole
kernel is paced by it and every other engine hides underneath:

  - Scores: 2-head row-tiled matmul pairs (tile_position (0,0)/(64,0))
    into one [128, 1024] f32 PSUM pair-tile (h0 cols 0:512 -> bank A,
    h1 cols 512:1024 -> bank B), double-buffered. One exp per kt step
    covers both heads.
  - b_k is dropped: softmax is invariant to per-q constants, and the
    only score term that varies over k is (q + b_q)@k. b_q is added to
    qT at evacuation (exact same math as the reference).
  - Softmax denominators: av row 64 (ones-row trick) -> DVE reciprocal
    [1, 512] -> gpsimd partition_broadcast -> attnT is normalized once
    at evacuation. Stage D needs no per-unit scaling.
  - Stage D per (tt, half): two concurrent row-tiled MMs -> one DVE
    tensor_add (cast fp16) -> DMA out.
  - Stage B of batch bi+1 (projections) is interleaved into stage C of
    bi via a work queue, so ACT never idles at batch boundaries.

PSUM: stp 2x2 banks + av 2x1 + scratch 2x1 (po pairs / ppr / pv) = 8.
"""

import numpy as np

import concourse.bacc as bacc
import concourse.mybir as mybir
from concourse.tile import TileContext
from concourse import bass_utils

dt = mybir.dt
F32 = dt.float32
F16 = dt.float16
AF = mybir.ActivationFunctionType
ALU = mybir.AluOpType

B, S, D = 4, 2048, 1024
H, DH = 16, 64
NCORES = 8
HPC = H // NCORES          # heads per core = 2
DHC = HPC * DH             # 128 projection cols per core

_CACHE = {}


def build_nc(b=B, s=S):
    d = D
    n_tt = s // 128            # 128-token tiles (k tiles, and q out-tiles)
    n_kt = d // 128            # contraction tiles for projections
    qw = 512                   # q-chunk width in stage C
    n_jc = s // qw
    assert s % 512 == 0 and d == 1024

    nc = bacc.Bacc("TRN2", target_bir_lowering=False, debug=False)

    x_d = nc.dram_tensor("x", [b, d, s], F16, kind="ExternalInput")
    wq_d = nc.dram_tensor("wq", [d, DHC], F16, kind="ExternalInput")
    wk_d = nc.dram_tensor("wk", [d, DHC], F16, kind="ExternalInput")
    wv_d = nc.dram_tensor("wv", [d, DHC], F16, kind="ExternalInput")
    bq_d = nc.dram_tensor("bq", [DHC, 1], F32, kind="ExternalInput")
    wo_d = nc.dram_tensor("wo", [DHC, d], F16, kind="ExternalInput")
    out_d = nc.dram_tensor("out", [b, s, d], F16, kind="ExternalOutput")

    with TileContext(nc) as tc:
        with (
            tc.tile_pool(name="const", bufs=1) as cpool,
            tc.tile_pool(name="wpool", bufs=3 * n_kt) as wpool,
            tc.tile_pool(name="xt", bufs=2 * n_kt) as xt_pool,
            tc.tile_pool(name="qk", bufs=4) as qk_pool,
            tc.tile_pool(name="vt", bufs=2) as vt_pool,
            tc.tile_pool(name="at", bufs=2) as at_pool,
            tc.tile_pool(name="pexp", bufs=3) as pexp_pool,
            tc.tile_pool(name="small", bufs=8) as small,
            tc.tile_pool(name="osb", bufs=3) as osb_pool,
            tc.tile_pool(name="ps", bufs=1, space="PSUM") as pp,
        ):
            # ---- constants & weights ----
            ones_col = cpool.tile([128, 32], F16, tag="ones_col")
            nc.vector.memset(ones_col[:, :], 1.0)
            bq = cpool.tile([DHC, 1], F32, tag="bq")
            nc.sync.dma_start(out=bq[:, :], in_=bq_d[:, :])

            w16 = {}
            for name, dram in (("q", wq_d), ("k", wk_d), ("v", wv_d)):
                for kt in range(n_kt):
                    wt = wpool.tile([128, DHC], F16, tag="w",
                                    name=f"w_{name}{kt}")
                    nc.sync.dma_start(
                        out=wt[:, :], in_=dram[kt * 128:(kt + 1) * 128, :]
                    )
                    w16[(name, kt)] = wt
            wo = cpool.tile([DHC, d], F16, tag="wo")
            nc.sync.dma_start(out=wo[:, :], in_=wo_d[:, :])

            # ---------- stage B builders (projections for one batch) ----
            def xT_dma(bi, xT):
                for kt in range(n_kt):
                    nc.sync.dma_start(
                        out=xT[kt][:, :],
                        in_=x_d[bi, kt * 128:(kt + 1) * 128, :],
                    )

            def qk_burst(name, c, xT, dst):
                """One 512-col chunk of the Q^T or K^T projection."""
                cs = slice(c * 512, (c + 1) * 512)
                ppr = pp.tile([128, 512], F32, tag="scr", bufs=2, name="ppr")
                for kt in range(n_kt):
                    nc.tensor.matmul(
                        ppr[:, :], w16[(name, kt)][:, :], xT[kt][:, cs],
                        start=(kt == 0), stop=(kt == n_kt - 1),
                    )
                if name == "q":
                    nc.vector.tensor_scalar_add(dst[:, cs], ppr[:, :],
                                                bq[:, 0:1])
                else:
                    nc.vector.tensor_copy(dst[:, cs], ppr[:, :])

            def v_burst(tt, xT, vt):
                """One 128-token tile of V in natural [tok, dh] layout."""
                pv = pp.tile([128, 128], F32, tag="scr", bufs=2, name="pv")
                for kt in range(n_kt):
                    nc.tensor.matmul(
                        pv[:, :], xT[kt][:, tt * 128:(tt + 1) * 128],
                        w16[("v", kt)][:, :],
                        start=(kt == 0), stop=(kt == n_kt - 1),
                    )
                vdst = vt.rearrange("p (t two sv) -> p t two sv",
                                    two=2, sv=65)[:, tt, :, 0:64]
                nc.vector.tensor_copy(
                    vdst, pv.rearrange("p (two sv) -> p two sv", two=2)
                )

            def ones_fill(vt):
                ones_dst = vt.rearrange("p (t two sv) -> p t two sv",
                                        two=2, sv=65)[:, :, :, 64]
                nc.vector.tensor_copy(
                    ones_dst,
                    ones_col[:, 0:2 * n_tt]
                    .rearrange("p (t two) -> p t two", two=2))

            def make_b_work(xT, qT, kT, vt):
                work = []
                for name, dst in (("q", qT), ("k", kT)):
                    for c in range(s // 512):
                        work.append(lambda n=name, c=c, xT=xT, dst=dst:
                                    qk_burst(n, c, xT, dst))
                for tt in range(n_tt):
                    work.append(lambda tt=tt, xT=xT, vt=vt:
                                v_burst(tt, xT, vt))
                return work

            # ---------- stage D (output projection units) ----------
            # d_stage holds the most recent q-chunk's units; they move to
            # d_queue one jc later so the PE never waits on the (slow-ish)
            # reciprocal -> broadcast -> normalize evacuation chain.
            d_queue = []   # (attnT, bi, tt, half)
            d_stage = []

            def emit_d_unit():
                if not d_queue:
                    return
                attnT_p, bi_out, tt, half = d_queue.pop(0)
                cs = slice(half * 512, (half + 1) * 512)
                po = pp.tile([128, 512], F32, tag="scr", bufs=2, name="po")
                nc.tensor.matmul(
                    po[:, :], attnT_p[:, tt * 128:(tt + 1) * 128],
                    wo[:, cs], start=True, stop=True,
                )
                osb = osb_pool.tile([128, 512], F16, tag="osb", name="osb")
                nc.vector.tensor_copy(osb[:, :], po[:, :])
                nc.sync.dma_start(
                    out=out_d[bi_out, tt * 128:(tt + 1) * 128, cs],
                    in_=osb[:, :],
                )

            # ---------- prologue: batch 0 stage A + leading stage B ----------
            def new_bufs():
                xT = [xt_pool.tile([128, s], F16, tag="xt", name=f"xT{kt}")
                      for kt in range(n_kt)]
                qT = qk_pool.tile([DHC, s], F16, tag="qT")
                kT = qk_pool.tile([DHC, s], F16, tag="kT")
                vt = vt_pool.tile([128, n_tt * 130], F16, tag="vt")
                return xT, qT, kT, vt

            cur = new_bufs()
            xT_dma(0, cur[0])
            ones_fill(cur[3])
            # minimum stage B to start jc=0: k/q chunk 0 and V tiles 0..9;
            # the rest streams in through b0_slots during jc0/jc1
            qk_burst("k", 0, cur[0], cur[2])
            qk_burst("q", 0, cur[0], cur[1])
            for tt in range(10):
                v_burst(tt, cur[0], cur[3])
            b0_slots = {}
            _b0 = [("k", 1), ("v", 10), ("k", 2), ("v", 11), ("k", 3),
                   ("q", 1)]
            for i, item in enumerate(_b0):
                b0_slots[2 * i + 1] = [item]
            b0_slots[13] = [("v", 12), ("v", 13)]
            b0_slots[15] = [("v", 14), ("v", 15)]
            b0_slots[17] = [("q", 2)]
            b0_slots[19] = [("q", 3)]

            def emit_b0(item):
                kind, i = item
                if kind == "v":
                    v_burst(i, cur[0], cur[3])
                else:
                    qk_burst(kind, i, cur[0],
                             cur[2] if kind == "k" else cur[1])

            # ---------- main loop ----------
            NSTEP = n_tt + 3    # kt pipeline depth: scores -> exp -> av lag 3
            for bi in range(b):
                xT, qT, kT, vt = cur
                if bi + 1 < b:
                    nxt = new_bufs()
                    b_work = make_b_work(*nxt)
                else:
                    nxt = None
                    b_work = []

                attnT = at_pool.tile([DHC, s], F16, tag="attnT")
                vtv = vt.rearrange("p (t two sv) -> p t two sv",
                                   two=2, sv=65)

                for jc in range(n_jc):
                    qs = slice(jc * qw, (jc + 1) * qw)
                    av = [pp.tile([65, qw], F32, tag="av", bufs=2,
                                  name=f"av{h}") for h in range(HPC)]
                    stps = {}
                    pexps = {}
                    for kt in range(NSTEP):
                        step = jc * NSTEP + kt
                        if kt < n_tt:
                            # score pair: h0 -> cols 0:512 (bank A),
                            # h1 -> cols 512:1024 (bank B)
                            stp = pp.tile([128, 2 * qw], F32, tag="stp",
                                          bufs=2, name="stp")
                            ks = slice(kt * 128, (kt + 1) * 128)
                            nc.tensor.matmul(
                                stp[:, 0:qw], kT[0:64, ks], qT[0:64, qs],
                                start=True, stop=True, tile_position=(0, 0),
                            )
                            nc.tensor.matmul(
                                stp[:, qw:2 * qw], kT[64:128, ks],
                                qT[64:128, qs],
                                start=True, stop=True, tile_position=(64, 0),
                            )
                            stps[kt] = stp
                        # interleave: stage D / stage B
                        if step % 2 == 0:
                            emit_d_unit()
                        else:
                            if step == 1 and nxt is not None:
                                xT_dma(bi + 1, nxt[0])
                                ones_fill(nxt[3])
                            if bi == 0 and step in b0_slots:
                                for item in b0_slots.pop(step):
                                    emit_b0(item)
                            elif step >= (21 if bi == 0 else 13) and b_work:
                                b_work.pop(0)()
                        if kt < n_tt:
                            # exp for both heads in one ACTIVATE
                            pexp = pexp_pool.tile([128, 2 * qw], F16,
                                                  tag="pexp", name="pexp",
                                                  bufs=5)
                            nc.scalar.activation(
                                pexp[:, :], stps[kt][:, :], AF.Exp,
                                scale=0.125,
                            )
                            pexps[kt] = pexp
                        if kt >= 3:
                            px = pexps.pop(kt - 3)
                            stps.pop(kt - 3, None)
                            for h in range(HPC):
                                nc.tensor.matmul(
                                    av[h][:, :], vtv[:, kt - 3, h, :],
                                    px[:, h * qw:(h + 1) * qw],
                                    start=(kt == 3), stop=(kt == NSTEP - 1),
                                )
                    # evacuate: normalize attnT by softmax denominators
                    # (av row 0 = ones-row result = sum of exp per q)
                    for h in range(HPC):
                        hs = slice(h * 64, (h + 1) * 64)
                        # stage row 64 to partition 0 (the custom recip op
                        # mishandles base_partition>0 inputs)
                        rrow = small.tile([1, qw], F32, tag="rrow",
                                          name="rrow")
                        nc.vector.tensor_copy(rrow[:, :], av[h][64:65, :])
                        recip = small.tile([1, qw], F32, tag="recip",
                                           name="recip")
                        nc.vector.reciprocal_approx_fast(recip[:, :],
                                                         rrow[:, :])
                        rb = small.tile([64, qw], F32, tag="rb", name="rb")
                        nc.gpsimd.partition_broadcast(rb[:, :], recip[0:1, :])
                        nc.vector.tensor_mul(attnT[hs, qs], av[h][0:64, :],
                                             rb[:, :])
                    # stage stage-D units for this q-chunk (4 tt x 2 halves);
                    # they become eligible one jc later
                    d_queue.extend(d_stage)
                    d_stage = []
                    for tt in range(jc * (qw // 128), (jc + 1) * (qw // 128)):
                        for half in range(2):
                            d_stage.append((attnT, bi, tt, half))

                # any stage-B stragglers for the next batch
                while b_work:
                    b_work.pop(0)()
                cur = nxt

            # flush remaining stage D
            d_queue.extend(d_stage)
            d_stage = []
            while d_queue:
                emit_d_unit()

    nc.compile()
    return nc


def _get_nc(b, s):
    key = (b, s)
    if key not in _CACHE:
        _CACHE[key] = build_nc(b, s)
    return _CACHE[key]


def make_in_maps(x, w_q, b_q, w_k, w_v, w_o):
    x16 = np.ascontiguousarray(
        np.asarray(x, dtype=np.float16).transpose(0, 2, 1))
    wq16 = np.asarray(w_q, dtype=np.float16)
    wk16 = np.asarray(w_k, dtype=np.float16)
    wv16 = np.asarray(w_v, dtype=np.float16)
    wo16 = np.asarray(w_o, dtype=np.float16)
    in_maps = []
    for i in range(NCORES):
        cs = slice(i * DHC, (i + 1) * DHC)
        in_maps.append({
            "x": x16,
            "wq": np.ascontiguousarray(wq16[:, cs]),
            "wk": np.ascontiguousarray(wk16[:, cs]),
            "wv": np.ascontiguousarray(wv16[:, cs]),
            "bq": np.ascontiguousarray(b_q[cs, None], dtype=np.float32),
            "wo": np.ascontiguousarray(wo16[cs, :]),
        })
    return in_maps


def kernel(x, w_q, b_q, w_k, b_k, w_v, b_v, w_o, b_o, _trace=False):
    x = np.asarray(x, dtype=np.float32)
    nc = _get_nc(x.shape[0], x.shape[1])
    in_maps = make_in_maps(x, w_q, b_q, w_k, w_v, w_o)
    kw = {}
    if _trace:
        import tempfile
        kw = dict(trace=True, trace_cores=list(range(NCORES)),
                  tmpdir=tempfile.mkdtemp(prefix="mha_trace_"))
    res = bass_utils.run_bass_kernel_spmd(
        nc, in_maps, core_ids=list(range(NCORES)), **kw
    )
    out = np.zeros(x.shape, dtype=np.float32)
    for i in range(NCORES):
        out += np.asarray(res.results[i]["out"], dtype=np.float32)
    out += np.asarray(b_o, dtype=np.float32)[None, None, :]
    # b_k cancels in softmax (constant per q); b_v @ w_o added here
    out += (np.asarray(b_v, dtype=np.float32)
            @ np.asarray(w_o, dtype=np.float32))[None, None, :]
    if _trace:
        return out, res
    return out


# revision 21
# speedup vs baseline: 1.3163x; 1.0164x over previous
"""Multi-head attention on 8 Trainium2 NeuronCores — ACT-paced fp16 pipeline.

Sharding: tensor-parallel over heads (2 heads/core), full batch on every
core; host sums the 8 partial outputs and adds b_o + b_v @ w_o.

Key structure (vs the previous version): the scalar engine's exp is the
irreducible floor (~73 us/batch at N=1024 per ACTIVATE), so the whole
kernel is paced by it and every other engine hides underneath:

  - Scores: 2-head row-tiled matmul pairs (tile_position (0,0)/(64,0))
    into one [128, 1024] f32 PSUM pair-tile (h0 cols 0:512 -> bank A,
    h1 cols 512:1024 -> bank B), double-buffered. One exp per kt step
    covers both heads.
  - b_k is dropped: softmax is invariant to per-q constants, and the
    only score term that varies over k is (q + b_q)@k. b_q is added to
    qT at evacuation (exact same math as the reference).
  - Softmax denominators: av row 64 (ones-row trick) -> DVE reciprocal
    [1, 512] -> gpsimd partition_broadcast -> attnT is normalized once
    at evacuation. Stage D needs no per-unit scaling.
  - Stage D per (tt, half): two concurrent row-tiled MMs -> one DVE
    tensor_add (cast fp16) -> DMA out.
  - Stage B of batch bi+1 (projections) is interleaved into stage C of
    bi via a work queue, so ACT never idles at batch boundaries.

PSUM: stp 2x2 banks + av 2x1 + scratch 2x1 (po pairs / ppr / pv) = 8.
"""

import numpy as np

import concourse.bacc as bacc
import concourse.mybir as mybir
from concourse.tile import TileContext
from concourse import bass_utils

dt = mybir.dt
F32 = dt.float32
F16 = dt.float16
AF = mybir.ActivationFunctionType
ALU = mybir.AluOpType

B, S, D = 4, 2048, 1024
H, DH = 16, 64
NCORES = 8
HPC = H // NCORES          # heads per core = 2
DHC = HPC * DH             # 128 projection cols per core

_CACHE = {}


def build_nc(b=B, s=S):
    d = D
    n_tt = s // 128            # 128-token tiles (k tiles, and q out-tiles)
    n_kt = d // 128            # contraction tiles for projections
    qw = 512                   # q-chunk width in stage C
    n_jc = s // qw
    assert s % 512 == 0 and d == 1024

    nc = bacc.Bacc("TRN2", target_bir_lowering=False, debug=False)

    x_d = nc.dram_tensor("x", [b, d, s], F16, kind="ExternalInput")
    wq_d = nc.dram_tensor("wq", [d, DHC], F16, kind="ExternalInput")
    wk_d = nc.dram_tensor("wk", [d, DHC], F16, kind="ExternalInput")
    wv_d = nc.dram_tensor("wv", [d, DHC], F16, kind="ExternalInput")
    bq_d = nc.dram_tensor("bq", [DHC, 1], F32, kind="ExternalInput")
    wo_d = nc.dram_tensor("wo", [DHC, d], F16, kind="ExternalInput")
    out_d = nc.dram_tensor("out", [b, s, d], F16, kind="ExternalOutput")

    with TileContext(nc) as tc:
        with (
            tc.tile_pool(name="const", bufs=1) as cpool,
            tc.tile_pool(name="wpool", bufs=3 * n_kt) as wpool,
            tc.tile_pool(name="xt", bufs=2 * n_kt) as xt_pool,
            tc.tile_pool(name="qk", bufs=4) as qk_pool,
            tc.tile_pool(name="vt", bufs=2) as vt_pool,
            tc.tile_pool(name="at", bufs=2) as at_pool,
            tc.tile_pool(name="pexp", bufs=3) as pexp_pool,
            tc.tile_pool(name="small", bufs=8) as small,
            tc.tile_pool(name="osb", bufs=3) as osb_pool,
            tc.tile_pool(name="ps", bufs=1, space="PSUM") as pp,
        ):
            # ---- constants & weights ----
            ones_col = cpool.tile([128, 32], F16, tag="ones_col")
            nc.vector.memset(ones_col[:, :], 1.0)
            bq = cpool.tile([DHC, 1], F32, tag="bq")
            nc.sync.dma_start(out=bq[:, :], in_=bq_d[:, :])

            w16 = {}
            for name, dram in (("q", wq_d), ("k", wk_d), ("v", wv_d)):
                for kt in range(n_kt):
                    wt = wpool.tile([128, DHC], F16, tag="w",
                                    name=f"w_{name}{kt}")
                    nc.sync.dma_start(
                        out=wt[:, :], in_=dram[kt * 128:(kt + 1) * 128, :]
                    )
                    w16[(name, kt)] = wt
            wo = cpool.tile([DHC, d], F16, tag="wo")
            nc.sync.dma_start(out=wo[:, :], in_=wo_d[:, :])

            # ---------- stage B builders (projections for one batch) ----
            def xT_dma(bi, xT):
                for kt in range(n_kt):
                    nc.sync.dma_start(
                        out=xT[kt][:, :],
                        in_=x_d[bi, kt * 128:(kt + 1) * 128, :],
                    )

            def qk_burst(name, c, xT, dst, w=512):
                """One w-col chunk of the Q^T or K^T projection."""
                cs = slice(c * w, (c + 1) * w)
                ppr = pp.tile([128, w], F32, tag="scr", bufs=2, name="ppr")
                for kt in range(n_kt):
                    nc.tensor.matmul(
                        ppr[:, :], w16[(name, kt)][:, :], xT[kt][:, cs],
                        start=(kt == 0), stop=(kt == n_kt - 1),
                    )
                if name == "q":
                    nc.vector.tensor_scalar_add(dst[:, cs], ppr[:, :],
                                                bq[:, 0:1])
                else:
                    nc.vector.tensor_copy(dst[:, cs], ppr[:, :])

            def v_burst(tt, xT, vt):
                """One 128-token tile of V in natural [tok, dh] layout."""
                pv = pp.tile([128, 128], F32, tag="scr", bufs=2, name="pv")
                for kt in range(n_kt):
                    nc.tensor.matmul(
                        pv[:, :], xT[kt][:, tt * 128:(tt + 1) * 128],
                        w16[("v", kt)][:, :],
                        start=(kt == 0), stop=(kt == n_kt - 1),
                    )
                vdst = vt.rearrange("p (t two sv) -> p t two sv",
                                    two=2, sv=65)[:, tt, :, 0:64]
                nc.vector.tensor_copy(
                    vdst, pv.rearrange("p (two sv) -> p two sv", two=2)
                )

            def ones_fill(vt):
                ones_dst = vt.rearrange("p (t two sv) -> p t two sv",
                                        two=2, sv=65)[:, :, :, 64]
                nc.vector.tensor_copy(
                    ones_dst,
                    ones_col[:, 0:2 * n_tt]
                    .rearrange("p (t two) -> p t two", two=2))

            def make_b_work(xT, qT, kT, vt):
                """Stage-B work items, ~0.9-1.3us of PE each, ordered so
                anything a batch's jc0 needs comes first (leftovers may
                spill into that batch's own early steps)."""
                work = []

                def qk(n, c, dst):
                    work.append(lambda: qk_burst(n, c, xT, dst, w=256))

                for c in range(4):
                    qk("k", c, kT)
                for c in range(4, 8):
                    qk("k", c, kT)
                qk("q", 0, qT)
                qk("q", 1, qT)
                for tt in range(n_tt):
                    work.append(lambda tt=tt: v_burst(tt, xT, vt))
                for c in range(2, 8):
                    qk("q", c, qT)
                return work

            # ---------- stage D (output projection units) ----------
            # d_stage holds the most recent q-chunk's units; they move to
            # d_queue one jc later so the PE never waits on the (slow-ish)
            # reciprocal -> broadcast -> normalize evacuation chain.
            d_queue = []   # (attnT, bi, tt, half)
            d_stage = []

            def emit_d_unit():
                if not d_queue:
                    return
                attnT_p, bi_out, tt, half = d_queue.pop(0)
                cs = slice(half * 512, (half + 1) * 512)
                po = pp.tile([128, 512], F32, tag="scr", bufs=2, name="po")
                nc.tensor.matmul(
                    po[:, :], attnT_p[:, tt * 128:(tt + 1) * 128],
                    wo[:, cs], start=True, stop=True,
                )
                osb = osb_pool.tile([128, 512], F16, tag="osb", name="osb")
                nc.vector.tensor_copy(osb[:, :], po[:, :])
                nc.sync.dma_start(
                    out=out_d[bi_out, tt * 128:(tt + 1) * 128, cs],
                    in_=osb[:, :],
                )

            # ---------- prologue: batch 0 stage A + leading stage B ----------
            def new_bufs():
                xT = [xt_pool.tile([128, s], F16, tag="xt", name=f"xT{kt}")
                      for kt in range(n_kt)]
                qT = qk_pool.tile([DHC, s], F16, tag="qT")
                kT = qk_pool.tile([DHC, s], F16, tag="kT")
                vt = vt_pool.tile([128, n_tt * 130], F16, tag="vt")
                return xT, qT, kT, vt

            cur = new_bufs()
            xT_dma(0, cur[0])
            ones_fill(cur[3])
            # minimum stage B to start jc=0: k/q chunk 0 and V tiles 0..2;
            # the rest streams in through b0_slots during jc0/jc1.
            # users: kT chunk c (512) at step 4c; V tile t at step t+3;
            # qT jc-chunk j at step 19j.
            qk_burst("k", 0, cur[0], cur[2])
            qk_burst("q", 0, cur[0], cur[1])
            for tt in range(3):
                v_burst(tt, cur[0], cur[3])
            _b0 = [("k", 1), ("v", 3), ("v", 4), ("v", 5), ("k", 2),
                   ("v", 6), ("v", 7), ("v", 8), ("k", 3), ("v", 9),
                   ("q", 1), ("v", 10), ("v", 11), ("v", 12), ("v", 13),
                   ("v", 14), ("v", 15), ("q", 2), ("q", 3)]
            b0_slots = {i + 1: [it] for i, it in enumerate(_b0)}

            def emit_b0(item):
                kind, i = item
                if kind == "v":
                    v_burst(i, cur[0], cur[3])
                else:
                    qk_burst(kind, i, cur[0],
                             cur[2] if kind == "k" else cur[1])

            # ---------- main loop ----------
            NSTEP = n_tt + 3    # kt pipeline depth: scores -> exp -> av lag 3
            for bi in range(b):
                xT, qT, kT, vt = cur
                if bi + 1 < b:
                    nxt = new_bufs()
                    b_work = make_b_work(*nxt)
                else:
                    nxt = None
                    b_work = []

                attnT = at_pool.tile([DHC, s], F16, tag="attnT")
                vtv = vt.rearrange("p (t two sv) -> p t two sv",
                                   two=2, sv=65)

                for jc in range(n_jc):
                    qs = slice(jc * qw, (jc + 1) * qw)
                    av = [pp.tile([65, qw], F32, tag="av", bufs=2,
                                  name=f"av{h}") for h in range(HPC)]
                    stps = {}
                    pexps = {}
                    for kt in range(NSTEP):
                        step = jc * NSTEP + kt
                        if kt < n_tt:
                            # score pair: h0 -> cols 0:512 (bank A),
                            # h1 -> cols 512:1024 (bank B)
                            stp = pp.tile([128, 2 * qw], F32, tag="stp",
                                          bufs=2, name="stp")
                            ks = slice(kt * 128, (kt + 1) * 128)
                            nc.tensor.matmul(
                                stp[:, 0:qw], kT[0:64, ks], qT[0:64, qs],
                                start=True, stop=True, tile_position=(0, 0),
                            )
                            nc.tensor.matmul(
                                stp[:, qw:2 * qw], kT[64:128, ks],
                                qT[64:128, qs],
                                start=True, stop=True, tile_position=(64, 0),
                            )
                            stps[kt] = stp
                        # interleave: stage D / stage B
                        if step == 1 and nxt is not None:
                            xT_dma(bi + 1, nxt[0])
                            ones_fill(nxt[3])
                        if bi == 0 and step in b0_slots:
                            for item in b0_slots.pop(step):
                                emit_b0(item)
                        elif step % 2 == 0 and d_queue:
                            emit_d_unit()
                        elif step >= 13 and b_work:
                            b_work.pop(0)()
                        if kt < n_tt:
                            # exp for both heads in one ACTIVATE
                            pexp = pexp_pool.tile([128, 2 * qw], F16,
                                                  tag="pexp", name="pexp",
                                                  bufs=5)
                            nc.scalar.activation(
                                pexp[:, :], stps[kt][:, :], AF.Exp,
                                scale=0.125,
                            )
                            pexps[kt] = pexp
                        if kt >= 3:
                            px = pexps.pop(kt - 3)
                            stps.pop(kt - 3, None)
                            for h in range(HPC):
                                nc.tensor.matmul(
                                    av[h][:, :], vtv[:, kt - 3, h, :],
                                    px[:, h * qw:(h + 1) * qw],
                                    start=(kt == 3), stop=(kt == NSTEP - 1),
                                )
                    # evacuate: normalize attnT by softmax denominators
                    # (av row 0 = ones-row result = sum of exp per q)
                    for h in range(HPC):
                        hs = slice(h * 64, (h + 1) * 64)
                        # stage row 64 to partition 0 (the custom recip op
                        # mishandles base_partition>0 inputs)
                        rrow = small.tile([1, qw], F32, tag="rrow",
                                          name="rrow")
                        nc.vector.tensor_copy(rrow[:, :], av[h][64:65, :])
                        recip = small.tile([1, qw], F32, tag="recip",
                                           name="recip")
                        nc.vector.reciprocal_approx_fast(recip[:, :],
                                                         rrow[:, :])
                        rb = small.tile([64, qw], F32, tag="rb", name="rb")
                        nc.gpsimd.partition_broadcast(rb[:, :], recip[0:1, :])
                        nc.vector.tensor_mul(attnT[hs, qs], av[h][0:64, :],
                                             rb[:, :])
                    # stage stage-D units for this q-chunk (4 tt x 2 halves);
                    # they become eligible one jc later (immediately on the
                    # last batch, to shrink the drain tail)
                    d_queue.extend(d_stage)
                    d_stage = []
                    units = [(attnT, bi, tt, half)
                             for tt in range(jc * (qw // 128),
                                             (jc + 1) * (qw // 128))
                             for half in range(2)]
                    if bi == b - 1:
                        d_queue.extend(units)
                    else:
                        d_stage = units

                # any stage-B stragglers for the next batch
                while b_work:
                    b_work.pop(0)()
                cur = nxt

            # flush remaining stage D
            d_queue.extend(d_stage)
            d_stage = []
            while d_queue:
                emit_d_unit()

    nc.compile()
    return nc


def _get_nc(b, s):
    key = (b, s)
    if key not in _CACHE:
        _CACHE[key] = build_nc(b, s)
    return _CACHE[key]


def make_in_maps(x, w_q, b_q, w_k, w_v, w_o):
    x16 = np.ascontiguousarray(
        np.asarray(x, dtype=np.float16).transpose(0, 2, 1))
    wq16 = np.asarray(w_q, dtype=np.float16)
    wk16 = np.asarray(w_k, dtype=np.float16)
    wv16 = np.asarray(w_v, dtype=np.float16)
    wo16 = np.asarray(w_o, dtype=np.float16)
    in_maps = []
    for i in range(NCORES):
        cs = slice(i * DHC, (i + 1) * DHC)
        in_maps.append({
            "x": x16,
            "wq": np.ascontiguousarray(wq16[:, cs]),
            "wk": np.ascontiguousarray(wk16[:, cs]),
            "wv": np.ascontiguousarray(wv16[:, cs]),
            "bq": np.ascontiguousarray(b_q[cs, None], dtype=np.float32),
            "wo": np.ascontiguousarray(wo16[cs, :]),
        })
    return in_maps


def kernel(x, w_q, b_q, w_k, b_k, w_v, b_v, w_o, b_o, _trace=False):
    x = np.asarray(x, dtype=np.float32)
    nc = _get_nc(x.shape[0], x.shape[1])
    in_maps = make_in_maps(x, w_q, b_q, w_k, w_v, w_o)
    kw = {}
    if _trace:
        import tempfile
        kw = dict(trace=True, trace_cores=list(range(NCORES)),
                  tmpdir=tempfile.mkdtemp(prefix="mha_trace_"))
    res = bass_utils.run_bass_kernel_spmd(
        nc, in_maps, core_ids=list(range(NCORES)), **kw
    )
    out = np.zeros(x.shape, dtype=np.float32)
    for i in range(NCORES):
        out += np.asarray(res.results[i]["out"], dtype=np.float32)
    out += np.asarray(b_o, dtype=np.float32)[None, None, :]
    # b_k cancels in softmax (constant per q); b_v @ w_o added here
    out += (np.asarray(b_v, dtype=np.float32)
            @ np.asarray(w_o, dtype=np.float32))[None, None, :]
    if _trace:
        return out, res
    return out


# revision 28
# speedup vs baseline: 1.3471x; 1.0234x over previous
"""Multi-head attention on 8 Trainium2 NeuronCores — ACT-paced fp16 pipeline.

Sharding: tensor-parallel over heads (2 heads/core), full batch on every
core; host sums the 8 partial outputs and adds b_o + b_v @ w_o.

Key structure (vs the previous version): the scalar engine's exp is the
irreducible floor (~73 us/batch at N=1024 per ACTIVATE), so the whole
kernel is paced by it and every other engine hides underneath:

  - Scores: 2-head row-tiled matmul pairs (tile_position (0,0)/(64,0))
    into one [128, 1024] f32 PSUM pair-tile (h0 cols 0:512 -> bank A,
    h1 cols 512:1024 -> bank B), double-buffered. One exp per kt step
    covers both heads.
  - b_k is dropped: softmax is invariant to per-q constants, and the
    only score term that varies over k is (q + b_q)@k. b_q is added to
    qT at evacuation (exact same math as the reference).
  - Softmax denominators: av row 64 (ones-row trick) -> DVE reciprocal
    [1, 512] -> gpsimd partition_broadcast -> attnT is normalized once
    at evacuation. Stage D needs no per-unit scaling.
  - Stage D per (tt, half): two concurrent row-tiled MMs -> one DVE
    tensor_add (cast fp16) -> DMA out.
  - Stage B of batch bi+1 (projections) is interleaved into stage C of
    bi via a work queue, so ACT never idles at batch boundaries.

PSUM: stp 2x2 banks + av 2x1 + scratch 2x1 (po pairs / ppr / pv) = 8.
"""

import numpy as np

import concourse.bacc as bacc
import concourse.mybir as mybir
from concourse.tile import TileContext
from concourse import bass_utils

dt = mybir.dt
F32 = dt.float32
F16 = dt.float16
AF = mybir.ActivationFunctionType
ALU = mybir.AluOpType

B, S, D = 4, 2048, 1024
H, DH = 16, 64
NCORES = 8
HPC = H // NCORES          # heads per core = 2
DHC = HPC * DH             # 128 projection cols per core

_CACHE = {}


def build_nc(b=B, s=S):
    d = D
    n_tt = s // 128            # 128-token tiles (k tiles, and q out-tiles)
    n_kt = d // 128            # contraction tiles for projections
    qw = 512                   # q-chunk width in stage C
    n_jc = s // qw
    assert s % 512 == 0 and d == 1024

    nc = bacc.Bacc("TRN2", target_bir_lowering=False, debug=False)

    x_d = nc.dram_tensor("x", [b, d, s], F16, kind="ExternalInput")
    wq_d = nc.dram_tensor("wq", [d, DHC], F16, kind="ExternalInput")
    wk_d = nc.dram_tensor("wk", [d, DHC], F16, kind="ExternalInput")
    wv_d = nc.dram_tensor("wv", [d, DHC], F16, kind="ExternalInput")
    bq_d = nc.dram_tensor("bq", [DHC, 1], F32, kind="ExternalInput")
    wo_d = nc.dram_tensor("wo", [DHC, d], F16, kind="ExternalInput")
    out_d = nc.dram_tensor("out", [b, s, d], F16, kind="ExternalOutput")

    with TileContext(nc) as tc:
        with (
            tc.tile_pool(name="const", bufs=1) as cpool,
            tc.tile_pool(name="wpool", bufs=3 * n_kt) as wpool,
            tc.tile_pool(name="xt", bufs=2 * n_kt) as xt_pool,
            tc.tile_pool(name="qk", bufs=4) as qk_pool,
            tc.tile_pool(name="vt", bufs=2) as vt_pool,
            tc.tile_pool(name="at", bufs=2) as at_pool,
            tc.tile_pool(name="pexp", bufs=3) as pexp_pool,
            tc.tile_pool(name="stg", bufs=4) as stg_pool,
            tc.tile_pool(name="small", bufs=8) as small,
            tc.tile_pool(name="osb", bufs=3) as osb_pool,
            tc.tile_pool(name="ps", bufs=1, space="PSUM") as pp,
        ):
            # ---- constants & weights ----
            ones_col = cpool.tile([128, 32], F16, tag="ones_col")
            nc.vector.memset(ones_col[:, :], 1.0)
            bq = cpool.tile([DHC, 1], F32, tag="bq")
            nc.sync.dma_start(out=bq[:, :], in_=bq_d[:, :])

            w16 = {}
            for name, dram in (("q", wq_d), ("k", wk_d), ("v", wv_d)):
                for kt in range(n_kt):
                    wt = wpool.tile([128, DHC], F16, tag="w",
                                    name=f"w_{name}{kt}")
                    nc.sync.dma_start(
                        out=wt[:, :], in_=dram[kt * 128:(kt + 1) * 128, :]
                    )
                    w16[(name, kt)] = wt
            wo = cpool.tile([DHC, d], F16, tag="wo")
            nc.sync.dma_start(out=wo[:, :], in_=wo_d[:, :])

            # ---------- stage B builders (projections for one batch) ----
            def xT_dma(bi, xT):
                for kt in range(n_kt):
                    nc.sync.dma_start(
                        out=xT[kt][:, :],
                        in_=x_d[bi, kt * 128:(kt + 1) * 128, :],
                    )

            def qk_burst(name, c, xT, dst, w=512):
                """One w-col chunk of the Q^T or K^T projection."""
                cs = slice(c * w, (c + 1) * w)
                ppr = pp.tile([128, w], F32, tag="scr", bufs=2, name="ppr")
                for kt in range(n_kt):
                    nc.tensor.matmul(
                        ppr[:, :], w16[(name, kt)][:, :], xT[kt][:, cs],
                        start=(kt == 0), stop=(kt == n_kt - 1),
                    )
                if name == "q":
                    nc.vector.tensor_scalar_add(dst[:, cs], ppr[:, :],
                                                bq[:, 0:1])
                else:
                    nc.vector.tensor_copy(dst[:, cs], ppr[:, :])

            def v_burst(tt, xT, vt):
                """One 128-token tile of V in natural [tok, dh] layout."""
                pv = pp.tile([128, 128], F32, tag="scr", bufs=2, name="pv")
                for kt in range(n_kt):
                    nc.tensor.matmul(
                        pv[:, :], xT[kt][:, tt * 128:(tt + 1) * 128],
                        w16[("v", kt)][:, :],
                        start=(kt == 0), stop=(kt == n_kt - 1),
                    )
                vdst = vt.rearrange("p (t two sv) -> p t two sv",
                                    two=2, sv=65)[:, tt, :, 0:64]
                nc.vector.tensor_copy(
                    vdst, pv.rearrange("p (two sv) -> p two sv", two=2)
                )

            def ones_fill(vt):
                ones_dst = vt.rearrange("p (t two sv) -> p t two sv",
                                        two=2, sv=65)[:, :, :, 64]
                nc.vector.tensor_copy(
                    ones_dst,
                    ones_col[:, 0:2 * n_tt]
                    .rearrange("p (t two) -> p t two", two=2))

            def make_b_work(xT, qT, kT, vt):
                """Stage-B work items, ~0.9-1.3us of PE each, ordered so
                anything a batch's jc0 needs comes first (leftovers may
                spill into that batch's own early steps)."""
                work = []

                def qk(n, c, dst):
                    work.append(lambda: qk_burst(n, c, xT, dst, w=256))

                for c in range(4):
                    qk("k", c, kT)
                for c in range(4, 8):
                    qk("k", c, kT)
                qk("q", 0, qT)
                qk("q", 1, qT)
                for tt in range(n_tt):
                    work.append(lambda tt=tt: v_burst(tt, xT, vt))
                for c in range(2, 8):
                    qk("q", c, qT)
                return work

            # ---------- stage D (output projection units) ----------
            # d_stage holds the most recent q-chunk's units; they move to
            # d_queue one jc later so the PE never waits on the (slow-ish)
            # reciprocal -> broadcast -> normalize evacuation chain.
            d_queue = []   # (attnT, bi, tt, half)
            d_stage = []

            def emit_d_unit():
                if not d_queue:
                    return
                attnT_p, bi_out, tt, half = d_queue.pop(0)
                cs = slice(half * 512, (half + 1) * 512)
                po = pp.tile([128, 512], F32, tag="scr", bufs=2, name="po")
                nc.tensor.matmul(
                    po[:, :], attnT_p[:, tt * 128:(tt + 1) * 128],
                    wo[:, cs], start=True, stop=True,
                )
                osb = osb_pool.tile([128, 512], F16, tag="osb", name="osb")
                nc.vector.tensor_copy(osb[:, :], po[:, :])
                nc.sync.dma_start(
                    out=out_d[bi_out, tt * 128:(tt + 1) * 128, cs],
                    in_=osb[:, :],
                )

            # ---------- prologue: batch 0 stage A + leading stage B ----------
            def new_bufs():
                xT = [xt_pool.tile([128, s], F16, tag="xt", name=f"xT{kt}")
                      for kt in range(n_kt)]
                qT = qk_pool.tile([DHC, s], F16, tag="qT")
                kT = qk_pool.tile([DHC, s], F16, tag="kT")
                vt = vt_pool.tile([128, n_tt * 130], F16, tag="vt")
                return xT, qT, kT, vt

            cur = new_bufs()
            xT_dma(0, cur[0])
            ones_fill(cur[3])
            # minimum stage B to start jc=0: k/q chunk 0 and V tiles 0..2;
            # the rest streams in through b0_slots during jc0/jc1.
            # users: kT chunk c (512) at step 4c; V tile t at step t+3;
            # qT jc-chunk j at step 19j.
            qk_burst("k", 0, cur[0], cur[2])
            qk_burst("q", 0, cur[0], cur[1])
            for tt in range(3):
                v_burst(tt, cur[0], cur[3])
            _b0 = [("k", 1), ("v", 3), ("v", 4), ("v", 5), ("k", 2),
                   ("v", 6), ("v", 7), ("v", 8), ("k", 3), ("v", 9),
                   ("q", 1), ("v", 10), ("v", 11), ("v", 12), ("v", 13),
                   ("v", 14), ("v", 15), ("q", 2), ("q", 3)]
            b0_slots = {i + 1: [it] for i, it in enumerate(_b0)}

            def emit_b0(item):
                kind, i = item
                if kind == "v":
                    v_burst(i, cur[0], cur[3])
                else:
                    qk_burst(kind, i, cur[0],
                             cur[2] if kind == "k" else cur[1])

            # ---------- main loop: one continuous score-step stream ----------
            # Global step G runs (bi, jc, kt) = (G//64, (G%64)//16, G%16).
            # Scores for chunk jc+1 (or the next batch) start immediately
            # after chunk jc's - no pipeline drain at chunk boundaries, so
            # the scalar engine's exp stream never runs dry. AV matmuls lag
            # LAG steps behind; the av banks are released by a single CAST
            # into an SBUF staging tile, and normalization happens off the
            # critical path (recip -> gpsimd broadcast -> multiply).
            LAG = 2
            SPB = n_jc * n_tt          # score steps per batch = 64
            NG = b * SPB
            bufs_of = {0: cur}
            work_now = []              # leftovers for the current batch
            work_next = []             # stage B of the next batch (gated)
            stps = {}
            pexps = {}
            avs = {}

            def evac_jc(bi_, jc_):
                """Release av banks and queue stage D for (bi_, jc_)."""
                attnT = attnTs[bi_]
                qs = slice(jc_ * qw, (jc_ + 1) * qw)
                for h in range(HPC):
                    hs = slice(h * 64, (h + 1) * 64)
                    av = avs.pop((bi_, jc_, h))
                    stg = stg_pool.tile([65, qw], F32, tag="stg",
                                        name="stg")
                    nc.vector.tensor_copy(stg[:, :], av[:, :])
                    # off-path: normalize into attnT
                    rrow = small.tile([1, qw], F32, tag="rrow", name="rrow")
                    nc.vector.tensor_copy(rrow[:, :], stg[64:65, :])
                    recip = small.tile([1, qw], F32, tag="recip",
                                       name="recip")
                    nc.vector.reciprocal_approx_fast(recip[:, :], rrow[:, :])
                    rb = small.tile([64, qw], F32, tag="rb", name="rb")
                    nc.gpsimd.partition_broadcast(rb[:, :], recip[0:1, :])
                    nc.vector.tensor_mul(attnT[hs, qs], stg[0:64, :],
                                         rb[:, :])
                units = [(attnT, bi_, tt, half)
                         for tt in range(jc_ * (qw // 128),
                                         (jc_ + 1) * (qw // 128))
                         for half in range(2)]
                d_queue.extend(d_stage)
                d_stage.clear()
                if bi_ == b - 1:
                    d_queue.extend(units)
                else:
                    d_stage.extend(units)

            attnTs = {}
            for G in range(NG + LAG):
                if G < NG:
                    bi, rem = divmod(G, SPB)
                    jc, kt = divmod(rem, n_tt)
                    if rem == 0:
                        # batch bookkeeping
                        work_now = work_next
                        work_next = []
                        if bi + 1 < b:
                            bufs_of[bi + 1] = new_bufs()
                            work_next = make_b_work(*bufs_of[bi + 1])
                        attnTs[bi] = at_pool.tile([DHC, s], F16, tag="attnT",
                                                  name="attnT")
                    xT, qT, kT, vt = bufs_of[bi]
                    if kt == 0:
                        for h in range(HPC):
                            avs[(bi, jc, h)] = pp.tile(
                                [65, qw], F32, tag="av", bufs=2,
                                name=f"av{h}")
                    qs = slice(jc * qw, (jc + 1) * qw)
                    # score pair: h0 -> cols 0:512 (bank A), h1 -> B
                    stp = pp.tile([128, 2 * qw], F32, tag="stp",
                                  bufs=2, name="stp")
                    ks = slice(kt * 128, (kt + 1) * 128)
                    nc.tensor.matmul(
                        stp[:, 0:qw], kT[0:64, ks], qT[0:64, qs],
                        start=True, stop=True, tile_position=(0, 0),
                    )
                    nc.tensor.matmul(
                        stp[:, qw:2 * qw], kT[64:128, ks], qT[64:128, qs],
                        start=True, stop=True, tile_position=(64, 0),
                    )
                    stps[G] = stp
                    # interleave: DMA prefetch / stage D / stage B
                    if rem == 1 and bi + 1 < b:
                        xT_dma(bi + 1, bufs_of[bi + 1][0])
                        ones_fill(bufs_of[bi + 1][3])
                    if bi == 0 and rem in b0_slots:
                        for item in b0_slots.pop(rem):
                            emit_b0(item)
                    elif work_now:
                        work_now.pop(0)()
                    elif rem % 2 == 0 and d_queue:
                        emit_d_unit()
                    elif rem >= 13 and work_next:
                        work_next.pop(0)()
                    # exp for both heads in one ACTIVATE
                    pexp = pexp_pool.tile([128, 2 * qw], F16, tag="pexp",
                                          name="pexp", bufs=LAG + 2)
                    nc.scalar.activation(
                        pexp[:, :], stps[G][:, :], AF.Exp, scale=0.125,
                    )
                    pexps[G] = pexp
                Gp = G - LAG
                if Gp >= 0:
                    bip, remp = divmod(Gp, SPB)
                    jcp, ktp = divmod(remp, n_tt)
                    px = pexps.pop(Gp)
                    stps.pop(Gp, None)
                    vtv = bufs_of[bip][3].rearrange(
                        "p (t two sv) -> p t two sv", two=2, sv=65)
                    for h in range(HPC):
                        nc.tensor.matmul(
                            avs[(bip, jcp, h)][:, :], vtv[:, ktp, h, :],
                            px[:, h * qw:(h + 1) * qw],
                            start=(ktp == 0), stop=(ktp == n_tt - 1),
                        )
                    if ktp == n_tt - 1:
                        evac_jc(bip, jcp)
                        if remp == SPB - 1 and bip > 0:
                            bufs_of.pop(bip - 1, None)

            # flush remaining stage D
            d_queue.extend(d_stage)
            d_stage.clear()
            while d_queue:
                emit_d_unit()

    nc.compile()
    return nc


def _get_nc(b, s):
    key = (b, s)
    if key not in _CACHE:
        _CACHE[key] = build_nc(b, s)
    return _CACHE[key]


def make_in_maps(x, w_q, b_q, w_k, w_v, w_o):
    x16 = np.ascontiguousarray(
        np.asarray(x, dtype=np.float16).transpose(0, 2, 1))
    wq16 = np.asarray(w_q, dtype=np.float16)
    wk16 = np.asarray(w_k, dtype=np.float16)
    wv16 = np.asarray(w_v, dtype=np.float16)
    wo16 = np.asarray(w_o, dtype=np.float16)
    in_maps = []
    for i in range(NCORES):
        cs = slice(i * DHC, (i + 1) * DHC)
        in_maps.append({
            "x": x16,
            "wq": np.ascontiguousarray(wq16[:, cs]),
            "wk": np.ascontiguousarray(wk16[:, cs]),
            "wv": np.ascontiguousarray(wv16[:, cs]),
            "bq": np.ascontiguousarray(b_q[cs, None], dtype=np.float32),
            "wo": np.ascontiguousarray(wo16[cs, :]),
        })
    return in_maps


def kernel(x, w_q, b_q, w_k, b_k, w_v, b_v, w_o, b_o, _trace=False):
    x = np.asarray(x, dtype=np.float32)
    nc = _get_nc(x.shape[0], x.shape[1])
    in_maps = make_in_maps(x, w_q, b_q, w_k, w_v, w_o)
    kw = {}
    if _trace:
        import tempfile
        kw = dict(trace=True, trace_cores=list(range(NCORES)),
                  tmpdir=tempfile.mkdtemp(prefix="mha_trace_"))
    res = bass_utils.run_bass_kernel_spmd(
        nc, in_maps, core_ids=list(range(NCORES)), **kw
    )
    out = np.zeros(x.shape, dtype=np.float32)
    for i in range(NCORES):
        out += np.asarray(res.results[i]["out"], dtype=np.float32)
    out += np.asarray(b_o, dtype=np.float32)[None, None, :]
    # b_k cancels in softmax (constant per q); b_v @ w_o added here
    out += (np.asarray(b_v, dtype=np.float32)
            @ np.asarray(w_o, dtype=np.float32))[None, None, :]
    if _trace:
        return out, res
    return out


# revision 32
# speedup vs baseline: 1.3731x; 1.0193x over previous
"""Multi-head attention on 8 Trainium2 NeuronCores — ACT-paced fp16 pipeline.

Sharding: tensor-parallel over heads (2 heads/core), full batch on every
core; host sums the 8 partial outputs and adds b_o + b_v @ w_o.

Key structure (vs the previous version): the scalar engine's exp is the
irreducible floor (~73 us/batch at N=1024 per ACTIVATE), so the whole
kernel is paced by it and every other engine hides underneath:

  - Scores: 2-head row-tiled matmul pairs (tile_position (0,0)/(64,0))
    into one [128, 1024] f32 PSUM pair-tile (h0 cols 0:512 -> bank A,
    h1 cols 512:1024 -> bank B), double-buffered. One exp per kt step
    covers both heads.
  - b_k is dropped: softmax is invariant to per-q constants, and the
    only score term that varies over k is (q + b_q)@k. b_q is added to
    qT at evacuation (exact same math as the reference).
  - Softmax denominators: av row 64 (ones-row trick) -> DVE reciprocal
    [1, 512] -> gpsimd partition_broadcast -> attnT is normalized once
    at evacuation. Stage D needs no per-unit scaling.
  - Stage D per (tt, half): two concurrent row-tiled MMs -> one DVE
    tensor_add (cast fp16) -> DMA out.
  - Stage B of batch bi+1 (projections) is interleaved into stage C of
    bi via a work queue, so ACT never idles at batch boundaries.

PSUM: stp 2x2 banks + av 2x1 + scratch 2x1 (po pairs / ppr / pv) = 8.
"""

import numpy as np

import concourse.bacc as bacc
import concourse.mybir as mybir
from concourse.tile import TileContext
from concourse import bass_utils

dt = mybir.dt
F32 = dt.float32
F16 = dt.float16
AF = mybir.ActivationFunctionType
ALU = mybir.AluOpType

B, S, D = 4, 2048, 1024
H, DH = 16, 64
NCORES = 8
HPC = H // NCORES          # heads per core = 2
DHC = HPC * DH             # 128 projection cols per core

_CACHE = {}


def build_nc(b=B, s=S):
    d = D
    n_tt = s // 128            # 128-token tiles (k tiles, and q out-tiles)
    n_kt = d // 128            # contraction tiles for projections
    qw = 512                   # q-chunk width in stage C
    n_jc = s // qw
    assert s % 512 == 0 and d == 1024

    nc = bacc.Bacc("TRN2", target_bir_lowering=False, debug=False)

    x_d = nc.dram_tensor("x", [b, d, s], F16, kind="ExternalInput")
    wq_d = nc.dram_tensor("wq", [d, DHC], F16, kind="ExternalInput")
    wk_d = nc.dram_tensor("wk", [d, DHC], F16, kind="ExternalInput")
    wv_d = nc.dram_tensor("wv", [d, DHC], F16, kind="ExternalInput")
    bq_d = nc.dram_tensor("bq", [DHC, 1], F32, kind="ExternalInput")
    wo_d = nc.dram_tensor("wo", [DHC, d], F16, kind="ExternalInput")
    out_d = nc.dram_tensor("out", [b, s, d], F16, kind="ExternalOutput")

    with TileContext(nc) as tc:
        with (
            tc.tile_pool(name="const", bufs=1) as cpool,
            tc.tile_pool(name="wpool", bufs=3) as wpool,
            tc.tile_pool(name="xt", bufs=2 * n_kt) as xt_pool,
            tc.tile_pool(name="qk", bufs=4) as qk_pool,
            tc.tile_pool(name="vt", bufs=2) as vt_pool,
            tc.tile_pool(name="at", bufs=2) as at_pool,
            tc.tile_pool(name="pexp", bufs=3) as pexp_pool,
            tc.tile_pool(name="stg", bufs=4) as stg_pool,
            tc.tile_pool(name="small", bufs=8) as small,
            tc.tile_pool(name="osb", bufs=3) as osb_pool,
            tc.tile_pool(name="ps", bufs=1, space="PSUM") as pp,
        ):
            # ---- constants & weights ----
            ones_col = cpool.tile([128, 32], F16, tag="ones_col")
            nc.vector.memset(ones_col[:, :], 1.0)
            bq = cpool.tile([DHC, 1], F32, tag="bq")
            nc.sync.dma_start(out=bq[:, :], in_=bq_d[:, :])

            # one batched DMA per weight tensor (issue cost on the sync
            # queue is ~600ns each; per-kt loads would serialize ~15us
            # ahead of the x prefetch)
            w16 = {}
            for name, dram in (("q", wq_d), ("k", wk_d), ("v", wv_d)):
                wall = wpool.tile([128, n_kt * DHC], F16, tag="w",
                                  name=f"w_{name}")
                nc.sync.dma_start(
                    out=wall.rearrange("p (kt c) -> p kt c", kt=n_kt),
                    in_=dram.rearrange("(kt p) c -> p kt c", p=128),
                )
                for kt in range(n_kt):
                    w16[(name, kt)] = wall[:, kt * DHC:(kt + 1) * DHC]
            wo = cpool.tile([DHC, d], F16, tag="wo")
            nc.sync.dma_start(out=wo[:, :], in_=wo_d[:, :])

            # ---------- stage B builders (projections for one batch) ----
            def xT_dma(bi, xT):
                for kt in range(n_kt):
                    nc.sync.dma_start(
                        out=xT[kt][:, :],
                        in_=x_d[bi, kt * 128:(kt + 1) * 128, :],
                    )

            def qk_burst(name, c, xT, dst, w=512):
                """One w-col chunk of the Q^T or K^T projection."""
                cs = slice(c * w, (c + 1) * w)
                ppr = pp.tile([128, w], F32, tag="scr", bufs=2, name="ppr")
                for kt in range(n_kt):
                    nc.tensor.matmul(
                        ppr[:, :], w16[(name, kt)][:, :], xT[kt][:, cs],
                        start=(kt == 0), stop=(kt == n_kt - 1),
                    )
                if name == "q":
                    nc.vector.tensor_scalar_add(dst[:, cs], ppr[:, :],
                                                bq[:, 0:1])
                else:
                    nc.vector.tensor_copy(dst[:, cs], ppr[:, :])

            def v_burst(tt, xT, vt):
                """One 128-token tile of V in natural [tok, dh] layout."""
                pv = pp.tile([128, 128], F32, tag="scr", bufs=2, name="pv")
                for kt in range(n_kt):
                    nc.tensor.matmul(
                        pv[:, :], xT[kt][:, tt * 128:(tt + 1) * 128],
                        w16[("v", kt)][:, :],
                        start=(kt == 0), stop=(kt == n_kt - 1),
                    )
                vdst = vt.rearrange("p (t two sv) -> p t two sv",
                                    two=2, sv=65)[:, tt, :, 0:64]
                nc.vector.tensor_copy(
                    vdst, pv.rearrange("p (two sv) -> p two sv", two=2)
                )

            def ones_fill(vt):
                ones_dst = vt.rearrange("p (t two sv) -> p t two sv",
                                        two=2, sv=65)[:, :, :, 64]
                nc.vector.tensor_copy(
                    ones_dst,
                    ones_col[:, 0:2 * n_tt]
                    .rearrange("p (t two) -> p t two", two=2))

            def make_b_work(xT, qT, kT, vt):
                """Stage-B work items, ~0.9-1.3us of PE each, ordered so
                anything a batch's jc0 needs comes first (leftovers may
                spill into that batch's own early steps)."""
                work = []

                def qk(n, c, dst):
                    work.append(lambda: qk_burst(n, c, xT, dst, w=256))

                for c in range(4):
                    qk("k", c, kT)
                for c in range(4, 8):
                    qk("k", c, kT)
                qk("q", 0, qT)
                qk("q", 1, qT)
                for tt in range(n_tt):
                    work.append(lambda tt=tt: v_burst(tt, xT, vt))
                for c in range(2, 8):
                    qk("q", c, qT)
                return work

            # ---------- stage D (output projection units) ----------
            # d_stage holds the most recent q-chunk's units; they move to
            # d_queue one jc later so the PE never waits on the (slow-ish)
            # reciprocal -> broadcast -> normalize evacuation chain.
            d_queue = []   # (attnT, bi, tt, half)
            d_stage = []

            def emit_d_unit():
                if not d_queue:
                    return
                attnT_p, bi_out, tt, half = d_queue.pop(0)
                cs = slice(half * 512, (half + 1) * 512)
                po = pp.tile([128, 512], F32, tag="scr", bufs=2, name="po")
                nc.tensor.matmul(
                    po[:, :], attnT_p[:, tt * 128:(tt + 1) * 128],
                    wo[:, cs], start=True, stop=True,
                )
                osb = osb_pool.tile([128, 512], F16, tag="osb", name="osb")
                nc.vector.tensor_copy(osb[:, :], po[:, :])
                nc.sync.dma_start(
                    out=out_d[bi_out, tt * 128:(tt + 1) * 128, cs],
                    in_=osb[:, :],
                )

            # ---------- prologue: batch 0 stage A + leading stage B ----------
            def new_bufs():
                xT = [xt_pool.tile([128, s], F16, tag="xt", name=f"xT{kt}")
                      for kt in range(n_kt)]
                qT = qk_pool.tile([DHC, s], F16, tag="qT")
                kT = qk_pool.tile([DHC, s], F16, tag="kT")
                vt = vt_pool.tile([128, n_tt * 130], F16, tag="vt")
                return xT, qT, kT, vt

            cur = new_bufs()
            xT_dma(0, cur[0])
            ones_fill(cur[3])
            # minimum stage B to start jc=0: k/q chunk 0 and V tiles 0..2;
            # the rest streams in through b0_slots during jc0/jc1.
            # users: kT chunk c (512) at step 4c; V tile t at step t+3;
            # qT jc-chunk j at step 19j.
            qk_burst("k", 0, cur[0], cur[2])
            qk_burst("q", 0, cur[0], cur[1])
            for tt in range(3):
                v_burst(tt, cur[0], cur[3])
            _b0 = [("k", 1), ("v", 3), ("v", 4), ("v", 5), ("k", 2),
                   ("v", 6), ("v", 7), ("v", 8), ("k", 3), ("v", 9),
                   ("q", 1), ("v", 10), ("v", 11), ("v", 12), ("v", 13),
                   ("v", 14), ("v", 15), ("q", 2), ("q", 3)]
            b0_slots = {i + 1: [it] for i, it in enumerate(_b0)}

            def emit_b0(item):
                kind, i = item
                if kind == "v":
                    v_burst(i, cur[0], cur[3])
                else:
                    qk_burst(kind, i, cur[0],
                             cur[2] if kind == "k" else cur[1])

            # ---------- main loop: one continuous score-step stream ----------
            # Global step G runs (bi, jc, kt) = (G//64, (G%64)//16, G%16).
            # Scores for chunk jc+1 (or the next batch) start immediately
            # after chunk jc's - no pipeline drain at chunk boundaries, so
            # the scalar engine's exp stream never runs dry. AV matmuls lag
            # LAG steps behind; the av banks are released by a single CAST
            # into an SBUF staging tile, and normalization happens off the
            # critical path (recip -> gpsimd broadcast -> multiply).
            LAG = 2
            SPB = n_jc * n_tt          # score steps per batch = 64
            NG = b * SPB
            bufs_of = {0: cur}
            work_now = []              # leftovers for the current batch
            work_next = []             # stage B of the next batch (gated)
            stps = {}
            pexps = {}
            avs = {}

            def evac_jc(bi_, jc_):
                """Release av banks and queue stage D for (bi_, jc_)."""
                attnT = attnTs[bi_]
                qs = slice(jc_ * qw, (jc_ + 1) * qw)
                for h in range(HPC):
                    hs = slice(h * 64, (h + 1) * 64)
                    av = avs.pop((bi_, jc_, h))
                    stg = stg_pool.tile([65, qw], F32, tag="stg",
                                        name="stg")
                    nc.vector.tensor_copy(stg[:, :], av[:, :])
                    # off-path: normalize into attnT
                    rrow = small.tile([1, qw], F32, tag="rrow", name="rrow")
                    nc.vector.tensor_copy(rrow[:, :], stg[64:65, :])
                    recip = small.tile([1, qw], F32, tag="recip",
                                       name="recip")
                    nc.vector.reciprocal_approx_fast(recip[:, :], rrow[:, :])
                    rb = small.tile([64, qw], F32, tag="rb", name="rb")
                    nc.gpsimd.partition_broadcast(rb[:, :], recip[0:1, :])
                    nc.vector.tensor_mul(attnT[hs, qs], stg[0:64, :],
                                         rb[:, :])
                units = [(attnT, bi_, tt, half)
                         for tt in range(jc_ * (qw // 128),
                                         (jc_ + 1) * (qw // 128))
                         for half in range(2)]
                d_queue.extend(d_stage)
                d_stage.clear()
                if bi_ == b - 1:
                    d_queue.extend(units)
                else:
                    d_stage.extend(units)

            attnTs = {}
            for G in range(NG + LAG):
                if G < NG:
                    bi, rem = divmod(G, SPB)
                    jc, kt = divmod(rem, n_tt)
                    if rem == 0:
                        # batch bookkeeping
                        work_now = work_next
                        work_next = []
                        if bi + 1 < b:
                            bufs_of[bi + 1] = new_bufs()
                            work_next = make_b_work(*bufs_of[bi + 1])
                        attnTs[bi] = at_pool.tile([DHC, s], F16, tag="attnT",
                                                  name="attnT")
                    xT, qT, kT, vt = bufs_of[bi]
                    if kt == 0:
                        for h in range(HPC):
                            avs[(bi, jc, h)] = pp.tile(
                                [65, qw], F32, tag="av", bufs=2,
                                name=f"av{h}")
                    qs = slice(jc * qw, (jc + 1) * qw)
                    # score pair: h0 -> cols 0:512 (bank A), h1 -> B
                    stp = pp.tile([128, 2 * qw], F32, tag="stp",
                                  bufs=2, name="stp")
                    ks = slice(kt * 128, (kt + 1) * 128)
                    nc.tensor.matmul(
                        stp[:, 0:qw], kT[0:64, ks], qT[0:64, qs],
                        start=True, stop=True, tile_position=(0, 0),
                    )
                    nc.tensor.matmul(
                        stp[:, qw:2 * qw], kT[64:128, ks], qT[64:128, qs],
                        start=True, stop=True, tile_position=(64, 0),
                    )
                    stps[G] = stp
                    # interleave: DMA prefetch / stage D / stage B
                    if rem == 1 and bi + 1 < b:
                        xT_dma(bi + 1, bufs_of[bi + 1][0])
                        ones_fill(bufs_of[bi + 1][3])
                    if bi == 0 and rem in b0_slots:
                        for item in b0_slots.pop(rem):
                            emit_b0(item)
                    elif work_now:
                        work_now.pop(0)()
                    elif d_queue and (rem % 2 == 0 or bi == b - 1):
                        emit_d_unit()
                    elif rem >= 13 and work_next:
                        work_next.pop(0)()
                    # exp for both heads in one ACTIVATE
                    pexp = pexp_pool.tile([128, 2 * qw], F16, tag="pexp",
                                          name="pexp", bufs=LAG + 2)
                    nc.scalar.activation(
                        pexp[:, :], stps[G][:, :], AF.Exp, scale=0.125,
                    )
                    pexps[G] = pexp
                Gp = G - LAG
                if Gp >= 0:
                    bip, remp = divmod(Gp, SPB)
                    jcp, ktp = divmod(remp, n_tt)
                    px = pexps.pop(Gp)
                    stps.pop(Gp, None)
                    vtv = bufs_of[bip][3].rearrange(
                        "p (t two sv) -> p t two sv", two=2, sv=65)
                    for h in range(HPC):
                        nc.tensor.matmul(
                            avs[(bip, jcp, h)][:, :], vtv[:, ktp, h, :],
                            px[:, h * qw:(h + 1) * qw],
                            start=(ktp == 0), stop=(ktp == n_tt - 1),
                        )
                    if ktp == n_tt - 1:
                        evac_jc(bip, jcp)
                        if remp == SPB - 1 and bip > 0:
                            bufs_of.pop(bip - 1, None)

            # flush remaining stage D
            d_queue.extend(d_stage)
            d_stage.clear()
            while d_queue:
                emit_d_unit()

    nc.compile()
    return nc


def _get_nc(b, s):
    key = (b, s)
    if key not in _CACHE:
        _CACHE[key] = build_nc(b, s)
    return _CACHE[key]


def make_in_maps(x, w_q, b_q, w_k, w_v, w_o):
    x16 = np.ascontiguousarray(
        np.asarray(x, dtype=np.float16).transpose(0, 2, 1))
    wq16 = np.asarray(w_q, dtype=np.float16)
    wk16 = np.asarray(w_k, dtype=np.float16)
    wv16 = np.asarray(w_v, dtype=np.float16)
    wo16 = np.asarray(w_o, dtype=np.float16)
    in_maps = []
    for i in range(NCORES):
        cs = slice(i * DHC, (i + 1) * DHC)
        in_maps.append({
            "x": x16,
            "wq": np.ascontiguousarray(wq16[:, cs]),
            "wk": np.ascontiguousarray(wk16[:, cs]),
            "wv": np.ascontiguousarray(wv16[:, cs]),
            "bq": np.ascontiguousarray(b_q[cs, None], dtype=np.float32),
            "wo": np.ascontiguousarray(wo16[cs, :]),
        })
    return in_maps


def kernel(x, w_q, b_q, w_k, b_k, w_v, b_v, w_o, b_o, _trace=False):
    x = np.asarray(x, dtype=np.float32)
    nc = _get_nc(x.shape[0], x.shape[1])
    in_maps = make_in_maps(x, w_q, b_q, w_k, w_v, w_o)
    kw = {}
    if _trace:
        import tempfile
        kw = dict(trace=True, trace_cores=list(range(NCORES)),
                  tmpdir=tempfile.mkdtemp(prefix="mha_trace_"))
    res = bass_utils.run_bass_kernel_spmd(
        nc, in_maps, core_ids=list(range(NCORES)), **kw
    )
    out = np.zeros(x.shape, dtype=np.float32)
    for i in range(NCORES):
        out += np.asarray(res.results[i]["out"], dtype=np.float32)
    out += np.asarray(b_o, dtype=np.float32)[None, None, :]
    # b_k cancels in softmax (constant per q); b_v @ w_o added here
    out += (np.asarray(b_v, dtype=np.float32)
            @ np.asarray(w_o, dtype=np.float32))[None, None, :]
    if _trace:
        return out, res
    return out
